# revision 4
# baseline (speedup 1.0000x reference)
"""Trainium2 Bass kernel for a dense transformer block (nn_Block_83880711291003).

Full (unsharded) inputs in, full output out. 8 NeuronCores:
  core c -> batch b = c//2, parity p = c%2.
Each core computes LN1 + K/V over its batch's full 2048 tokens and owns 1024
query tokens (4 chunks of 256). A host-side chunk permutation places each
parity's own chunks at fixed x'-positions {1,3,5,7} so ONE SPMD program works
for both parities: attention slot s (span 4/8/12/16 kv-tiles over the x'
prefix) handles the own chunk at x' position 2s+1; causality is enforced by
host-built additive masks folded into the score PSUM via DoubleRow identity
matmuls. Heavy matmuls (QKV, AV, out-proj, FFN) run as fp8e4m3 DoubleRow
(256-deep contraction, 0.5 cycles/col); scores stay bf16.
"""

import sys

for _p in ("/opt/trn_rl_repo", "/root/.axon_site/_ro/trn_rl_repo"):
    if _p not in sys.path:
        sys.path.append(_p)

from contextlib import ExitStack

import ml_dtypes
import numpy as np

import concourse.bass as bass
import concourse.tile as tile
from concourse import mybir
from concourse.bass_utils import run_bass_kernel_spmd
from concourse.masks import make_identity
from concourse.vector_clock import ScopedClock

FP32 = mybir.dt.float32
BF16 = mybir.dt.bfloat16
FP8 = mybir.dt.float8e4
BFNP = ml_dtypes.bfloat16
F8NP = ml_dtypes.float8_e4m3

ACT = mybir.ActivationFunctionType
ALU = mybir.AluOpType
DR = mybir.MatmulPerfMode.DoubleRow

B, T, D = 4, 2048, 512
H, DK = 8, 64
FF = 4 * D
EPS = 1e-5
WS = 16.0          # fp8 weight scale
IWS = 1.0 / WS
NEG = -240.0       # additive mask value (fp8e4m3 max normal magnitude)
SPANS = (4, 8, 12, 16)      # kv span per slot, in 128-tiles
OWN_POS = (1, 3, 5, 7)      # x' chunk position of each slot's own q chunk
# per-parity x' chunk order: x' position i holds original chunk PERM[p][i]
PERM = ((1, 0, 2, 3, 5, 4, 6, 7), (0, 1, 3, 2, 4, 5, 7, 6))
OWN_T = 1024
OWN_NT = 8

# ---------------------------------------------------------------------------
# Workaround: this walrus build rejects >1 semaphore wait per instruction.
# ---------------------------------------------------------------------------
_uid = [0]


def _split_multi_waits(nc):
    for blk in nc.m.functions[0].blocks:
        insts = list(blk.instructions)
        out, changed = [], False
        for inst in insts:
            si = inst.sync_info
            waits = list(si.on_wait) if si else []
            if len(waits) > 1:
                changed = True
                for w in waits[:-1]:
                    _uid[0] += 1
                    nop = mybir.InstNoOp(name=f"I-waitfix-{_uid[0]}", ins=[], outs=[])
                    nop.engine = inst.engine
                    nop.sync_info = mybir.SyncInfo(on_wait=[w], on_update=[])
                    out.append(nop)
                inst.sync_info = mybir.SyncInfo(
                    on_wait=[waits[-1]], on_update=list(si.on_update)
                )
            out.append(inst)
        if changed:
            blk.instructions = out


def _patched_drain_and_barrier(self, tick_clock, wait_clock):
    nc = self.nc
    probe = nc.sync.nop()
    wait_clock.add_sem_waits(probe.ins, ScopedClock({None: tick_clock.global_clock}))
    nc.sync.drain()
    nc.all_engine_barrier()
    popped = nc._tile_sem_poison_stack.pop()
    assert popped is self._sem_poison
    nc.clear_and_free_semaphores(list(self.sems.allocated().values()))
    nc.all_engine_barrier()


tile.TileContext._drain_and_barrier = _patched_drain_and_barrier


# ---------------------------------------------------------------------------
# Device program (identical on all 8 cores)
# ---------------------------------------------------------------------------
import os as _os
CFG = {
    "ht_early": _os.environ.get("CFG_HT_EARLY", "act"),   # hT8 copies t<4
    "ht_mid": _os.environ.get("CFG_HT_MID", "act"),       # hT8 copies t>=4
    "h2t": _os.environ.get("CFG_H2T", "dve"),             # h2T8 copies
    "oq_late_act": _os.environ.get("CFG_OQ", "0") == "1", # oq s>=2 on ACT
    "kq0_act": _os.environ.get("CFG_KQ0", "1") == "1",    # kq g0/s0 on ACT
}


def _build_program():
    nc = bass.Bass("TRN2", target_bir_lowering=False, debug=False)

    din = {}
    for name, shape, dt in [
        ("xp", [T, D], FP32),                 # permuted x for this core
        ("wq8", [2, 128, 2, D], FP8),
        ("wk8", [2, 128, 2, D], FP8),
        ("wv8", [2, 128, 2, D], FP8),
        ("wp8", [2, 128, 2, D], FP8),         # [quad A/B, p, j, d]
        ("w18", [2, 128, 2, FF], FP8),
        ("w28", [8, 128, 2, D], FP8),
        ("bqk", [128, 8], FP32),              # cols 0-3 bq per pr; 4-7 bk
        ("b1s", [128, 16], FP32),             # WS * b1 per hidden tile
        ("bv_row", [1, D], FP32),
        ("bp_row", [1, D], FP32),
        ("b2_row", [1, D], FP32),
        ("masks8", [4, 128, 2, 1024], FP8),   # per slot: DR mask rhs (j1=0)
    ]:
        din[name] = nc.dram_tensor(name, shape, dt, kind="ExternalInput").ap()
    out_dram = nc.dram_tensor("out", [OWN_T, D], FP32, kind="ExternalOutput").ap()

    with tile.TileContext(nc) as tc, ExitStack() as ctx:
        P = ctx.enter_context

        wpool = P(tc.tile_pool(name="weights", bufs=1))
        persist = P(tc.tile_pool(name="persist", bufs=1))
        xio = P(tc.tile_pool(name="xio", bufs=5))
        small = P(tc.tile_pool(name="small", bufs=6))
        hpool = P(tc.tile_pool(name="htok", bufs=4))
        ppool = P(tc.tile_pool(name="pT", bufs=4))
        opool = P(tc.tile_pool(name="outio", bufs=3))
        f18pool = P(tc.tile_pool(name="f18", bufs=2))
        psS = P(tc.tile_pool(name="psS", bufs=2, space="PSUM"))   # [128,1024]
        psA = P(tc.tile_pool(name="psA", bufs=2, space="PSUM"))   # AV accum
        psW = P(tc.tile_pool(name="psW", bufs=2, space="PSUM"))   # [128,512]

        # ---- input DMAs: x tiles interleaved with weights (SP queue);
        # masks/broadcasts/FFN weights via gpsimd (SWDGE) ----
        x_t = [None] * 16
        own_slot = {2: 0, 3: 1, 6: 2, 7: 3, 10: 4, 11: 5, 14: 6, 15: 7}
        wq8 = [wpool.tile([128, 2, D], FP8, tag=f"wq8{k}", name=f"wq8{k}")
               for k in range(2)]
        wk8 = [wpool.tile([128, 2, D], FP8, tag=f"wk8{k}", name=f"wk8{k}")
               for k in range(2)]
        wv8 = [wpool.tile([128, 2, D], FP8, tag=f"wv8{k}", name=f"wv8{k}")
               for k in range(2)]
        wp8 = [wpool.tile([128, 2, D], FP8, tag=f"wp8{k}", name=f"wp8{k}")
               for k in range(2)]

        def dma_x(t):
            if t in own_slot:
                x_t[t] = persist.tile([128, D], FP32, tag=f"xo{t}", name=f"xo{t}")
            else:
                x_t[t] = xio.tile([128, D], FP32, tag="xin", name="xin")
            nc.sync.dma_start(x_t[t][:], din["xp"][t * 128:(t + 1) * 128, :])

        bqk = wpool.tile([128, 8], FP32, tag="bqk", name="bqk")
        b1s = wpool.tile([128, 16], FP32, tag="b1s", name="b1s")
        for t in range(4):
            dma_x(t)
        for k in range(2):
            nc.sync.dma_start(wv8[k][:], din["wv8"][k])
            nc.sync.dma_start(wk8[k][:], din["wk8"][k])
        for t in range(4, 8):
            dma_x(t)
        for k in range(2):
            nc.sync.dma_start(wq8[k][:], din["wq8"][k])
        nc.sync.dma_start(bqk[:], din["bqk"][:])
        for t in range(8, 16):
            dma_x(t)
        for k in range(2):
            nc.sync.dma_start(wp8[k][:], din["wp8"][k])
        nc.sync.dma_start(b1s[:], din["b1s"][:])

        # constants first: they gate the first transposes/LN ops and must not
        # queue behind SWDGE descriptor generation on the Pool engine
        ident8 = wpool.tile([128, 128], FP8, tag="ident8", name="ident8")
        make_identity(nc, ident8[:])
        identB = wpool.tile([128, 128], BF16, tag="identB", name="identB")
        make_identity(nc, identB[:])
        identDR = wpool.tile([128, 2, 128], FP8, tag="identDR", name="identDR")
        nc.vector.memset(identDR[:, 1, :], 0.0)
        make_identity(nc, identDR[:, 0, :])
        eps_t = wpool.tile([128, 1], FP32, tag="eps", name="eps")
        nc.vector.memset(eps_t[:], EPS)
        # preload ACT tables for Exp/Relu/Sqrt before the hot loops
        warm = wpool.tile([128, 1], FP32, tag="warm", name="warm")
        nc.scalar.activation(out=warm[:], in_=eps_t[:], func=ACT.Exp, scale=1.0)
        nc.scalar.activation(out=warm[:], in_=eps_t[:], func=ACT.Relu, scale=1.0)
        nc.scalar.activation(out=warm[:], in_=eps_t[:], func=ACT.Sqrt, scale=1.0)

        def bcast_row(name):
            t = wpool.tile([128, D], FP32, tag=f"bc_{name}", name=f"bc_{name}")
            src = din[name]
            ap = bass.AP(tensor=src.tensor, offset=src.offset,
                         ap=[[0, 128], src.ap[1]])
            nc.scalar.dma_start(out=t[:], in_=ap)
            return t

        bv_b = bcast_row("bv_row")
        bp_b = bcast_row("bp_row")
        b2_b = bcast_row("b2_row")

        masks8 = wpool.tile([128, 4, 2, 1024], FP8, tag="masks8", name="masks8")
        nc.sync.dma_start(masks8[:], din["masks8"].rearrange("s p j f -> p s j f"))
        w18 = [wpool.tile([128, 2, FF], FP8, tag=f"w18{k}", name=f"w18{k}")
               for k in range(2)]
        w28 = [wpool.tile([128, 2, D], FP8, tag=f"w28{k}", name=f"w28{k}")
               for k in range(8)]
        for k in range(2):
            nc.sync.dma_start(w18[k][:], din["w18"][k])
        for k in range(8):
            nc.sync.dma_start(w28[k][:], din["w28"][k])

        # ---- long-lived activations ----
        hT8 = [persist.tile([128, 2, T], FP8, tag=f"hT8{k}", name=f"hT8{k}")
               for k in range(2)]
        kT = [persist.tile([128, T], BF16, tag=f"kT{pr}", name=f"kT{pr}")
              for pr in range(4)]
        qT = [persist.tile([128, 4, 256], BF16, tag=f"qT{pr}", name=f"qT{pr}")
              for pr in range(4)]
        # v8[p, kvpair, j, h, k]; col 64 is the ones column (denominator)
        v8 = persist.tile([128, 8, 2, H, 65], FP8, tag="v8", name="v8")
        nc.vector.memset(v8[:, :, :, :, 64], 1.0)
        oq = [persist.tile([128, 2, OWN_T], FP8, tag=f"oq{g}", name=f"oq{g}")
              for g in range(2)]
        xb = [persist.tile([128, D], FP32, tag=f"xb{t}", name=f"xb{t}")
              for t in range(OWN_NT)]
        x2 = [persist.tile([128, D], FP32, tag=f"x2_{t}", name=f"x2_{t}")
              for t in range(OWN_NT)]
        h2T8 = [persist.tile([128, 2, OWN_T], FP8, tag=f"h2T8{k}", name=f"h2T8{k}")
                for k in range(2)]

        # LN statistics, batched: per-tile bn stats land in a shared [128,2,N]
        # tile; one Sqrt + one reciprocal per tile-group keeps ACT's queue
        # clear of per-tile LN work.
        mvs1 = persist.tile([128, 2, 16], FP32, tag="mvs1", name="mvs1")
        rst1 = persist.tile([128, 16], FP32, tag="rst1", name="rst1")
        mvs2 = persist.tile([128, 2, 8], FP32, tag="mvs2", name="mvs2")
        rst2 = persist.tile([128, 8], FP32, tag="rst2", name="rst2")

        def ln_stats(x_in, mvs, t):
            stats = small.tile([128, 6], FP32, tag="bnst", name="bnst")
            nc.vector.bn_stats(out=stats[:], in_=x_in[:])
            mv = small.tile([128, 2], FP32, tag="bnmv", name="bnmv")
            nc.vector.bn_aggr(out=mv[:], in_=stats[:])
            nc.vector.tensor_copy(mvs[:, :, t], mv[:])

        def ln_rstd(mvs, rst, t0, n):
            rs = small.tile([128, 4], FP32, tag="rs", name="rs")
            nc.scalar.activation(out=rs[:, 0:n], in_=mvs[:, 1, t0:t0 + n],
                                 func=ACT.Sqrt, bias=eps_t[:], scale=1.0)
            nc.vector.reciprocal(out=rst[:, t0:t0 + n], in_=rs[:, 0:n])

        def ln_apply(x_in, h_out, mvs, rst, t, eng=None):
            eng = eng or nc.vector
            eng.tensor_scalar(
                out=h_out[:], in0=x_in[:], scalar1=mvs[:, 0, t:t + 1],
                scalar2=rst[:, t:t + 1], op0=ALU.subtract, op1=ALU.mult)

        def transpose_pairs(h_t, dst, dst_col, n, copy_eng="mix"):
            """dst[k][:, j, dst_col:+n] = h_t[:, (2k+j)*128:+128].T
            (bf16 transpose through PSUM, cast to fp8 on the copy out)."""
            for k in range(2):
                ps = psW.tile([128, 2, 128], BF16, tag="w", name="w")
                for j in range(2):
                    nc.tensor.matmul(ps[:, j, :],
                                     h_t[:, (2 * k + j) * 128:(2 * k + j + 1) * 128],
                                     identB[:], is_transpose=True,
                                     start=(j == 0), stop=(j == 1),
                                     skip_group_check=True)
                if copy_eng == "act" or (copy_eng == "mix" and k == 1):
                    nc.scalar.copy(dst[k][:, :, dst_col:dst_col + n], ps[:])
                else:
                    nc.vector.tensor_copy(dst[k][:, :, dst_col:dst_col + n],
                                          ps[:])

        # ---------------- stage A: LN1 + QKV per tile/group ----------------
        def ln_tile(t):
            h_t = hpool.tile([128, D], BF16, tag="h1", name="h1")
            ln_apply(x_t[t], h_t, mvs1, rst1, t,
                     nc.gpsimd if t >= 4 else nc.vector)
            transpose_pairs(h_t, hT8, t * 128, 128,
                            CFG["ht_early"] if t < 4 else CFG["ht_mid"])
            if t in own_slot:
                nc.gpsimd.tensor_add(xb[own_slot[t]][:], x_t[t][:], bp_b[:])
            # V projection for this tile
            ps = psW.tile([128, D], FP32, tag="w", name="w")
            for k in range(2):
                nc.tensor.matmul(ps[:], hT8[k][:, :, t * 128:(t + 1) * 128],
                                 wv8[k][:], start=(k == 0), stop=(k == 1),
                                 perf_mode=DR)
            nc.vector.scalar_tensor_tensor(
                out=v8[:, t // 2, t % 2, :, 0:64],
                in0=ps[:].rearrange("p (h k) -> p h k", h=H),
                scalar=IWS,
                in1=bv_b[:].rearrange("p (h k) -> p h k", h=H),
                op0=ALU.mult, op1=ALU.add)

        def k_group(g, prs=range(4)):
            """kT for 512-token group g (x' tiles 4g..4g+3)."""
            for pr in prs:
                ps = psW.tile([128, D], FP32, tag="w", name="w")
                for k in range(2):
                    nc.tensor.matmul(
                        ps[:], wk8[k][:, :, pr * 128:(pr + 1) * 128],
                        hT8[k][:, :, g * 512:(g + 1) * 512],
                        start=(k == 0), stop=(k == 1), perf_mode=DR)
                if g == 0 and CFG["kq0_act"]:
                    nc.scalar.activation(
                        out=kT[pr][:, g * 512:(g + 1) * 512], in_=ps[:],
                        func=ACT.Identity, bias=bqk[:, 4 + pr:5 + pr],
                        scale=IWS)
                else:
                    nc.vector.tensor_scalar(
                        out=kT[pr][:, g * 512:(g + 1) * 512], in0=ps[:],
                        scalar1=IWS, scalar2=bqk[:, 4 + pr:5 + pr],
                        op0=ALU.mult, op1=ALU.add)

        def q_slot(s, prs=range(4)):
            pos = OWN_POS[s]
            for pr in prs:
                ps = psW.tile([128, 256], FP32, tag="w", name="w")
                for k in range(2):
                    nc.tensor.matmul(
                        ps[:], wq8[k][:, :, pr * 128:(pr + 1) * 128],
                        hT8[k][:, :, pos * 256:(pos + 1) * 256],
                        start=(k == 0), stop=(k == 1), perf_mode=DR)
                if s == 0 and CFG["kq0_act"]:
                    nc.scalar.activation(
                        out=qT[pr][:, s, :], in_=ps[:], func=ACT.Identity,
                        bias=bqk[:, pr:pr + 1], scale=IWS)
                else:
                    nc.vector.tensor_scalar(
                        out=qT[pr][:, s, :], in0=ps[:],
                        scalar1=IWS, scalar2=bqk[:, pr:pr + 1],
                        op0=ALU.mult, op1=ALU.add)

        # ---------------- attention (software-pipelined head pair) --------
        def att_scores(s, h, q):
            """scores + mask + exp for quad q of (slot s, head h) -> pt8."""
            nquads = SPANS[s] // 4
            pr, sub = h // 2, h % 2
            krows = kT[pr][sub * 64:(sub + 1) * 64, :]
            qrows = qT[pr][sub * 64:(sub + 1) * 64, s, :]
            sps = psS.tile([128, 4, 256], FP32, tag="sps", name="sps")
            masked = (q == nquads - 1)
            for j in range(4):
                u = q * 4 + j
                nc.tensor.matmul(sps[:, j, :],
                                 krows[:, u * 128:(u + 1) * 128], qrows,
                                 start=(j % 2 == 0),
                                 stop=(j % 2 == 1 and not masked),
                                 skip_group_check=True)
            if masked:
                for half in range(2):
                    nc.tensor.matmul(
                        sps[:, 2 * half:2 * half + 2, :].rearrange(
                            "p j f -> p (j f)"),
                        identDR[:],
                        masks8[:, s, :, half * 512:(half + 1) * 512],
                        start=False, stop=True, perf_mode=DR,
                        skip_group_check=True)
            pt8 = ppool.tile([128, 4, 256], FP8, tag="pt8", name="pt8")
            nc.scalar.activation(out=pt8[:], in_=sps[:], func=ACT.Exp,
                                 scale=0.125)
            return pt8

        def att_av(s, h, q, pt8, ops, oi):
            nquads = SPANS[s] // 4
            for jp in range(2):
                kvp = q * 2 + jp
                for sb in range(2):
                    nc.tensor.matmul(
                        ops[:, 2 * oi + sb, :],
                        pt8[:, 2 * jp:2 * jp + 2, sb * 128:(sb + 1) * 128],
                        v8[:, kvp, :, h, :],
                        start=(q == 0 and jp == 0 and oi == 0 and sb == 0),
                        stop=(q == nquads - 1 and jp == 1),
                        perf_mode=DR, skip_group_check=True)

        def att_head_pair(s, h0, weave=()):
            """heads h0, h0+1 with scores/AV software pipelining; `weave` is a
            list of small zero-arg emitters drained at stage boundaries."""
            nquads = SPANS[s] // 4
            weave = list(weave)
            ops = psA.tile([128, 4, 65], FP32, tag="ops", name="ops")
            pend = []
            for q in range(nquads):
                pt_a = att_scores(s, h0, q)
                pend.append((h0, q, pt_a, 0))
                if len(pend) > 1:
                    hh, qq, pt, oi = pend.pop(0)
                    att_av(s, hh, qq, pt, ops, oi)
                if weave:
                    weave.pop(0)()
                pt_b = att_scores(s, h0 + 1, q)
                pend.append((h0 + 1, q, pt_b, 1))
                hh, qq, pt, oi = pend.pop(0)
                att_av(s, hh, qq, pt, ops, oi)
                if weave:
                    weave.pop(0)()
            hh, qq, pt, oi = pend.pop(0)
            att_av(s, hh, qq, pt, ops, oi)
            for fn in weave:
                fn()
            rc = small.tile([128, 4], FP32, tag="rc", name="rc")
            nc.vector.reciprocal(out=rc[:], in_=ops[:, :, 64])
            for sb in range(2):
                tcol = (s * 2 + sb) * 128
                ps = psW.tile([128, 128], BF16, tag="w", name="w")
                for i in range(2):
                    oS = small.tile([128, 64], BF16, tag="oS", name="oS")
                    nc.vector.tensor_scalar(
                        out=oS[:], in0=ops[:, 2 * i + sb, 0:64],
                        scalar1=rc[:, 2 * i + sb:2 * i + sb + 1],
                        scalar2=None, op0=ALU.mult)
                    nc.tensor.transpose(ps[i * 64:(i + 1) * 64, :], oS[:],
                                        identB[:])
                if (s >= 2 and CFG["oq_late_act"]) or sb == 1:
                    nc.scalar.copy(
                        oq[h0 // 4][:, (h0 % 4) // 2, tcol:tcol + 128], ps[:])
                else:
                    nc.vector.tensor_copy(
                        oq[h0 // 4][:, (h0 % 4) // 2, tcol:tcol + 128], ps[:])

        # ---------------- per-own-tile epilogue: proj + LN2 ----------------
        def epilogue_proj(t):
            ps = psW.tile([128, D], FP32, tag="w", name="w")
            for g in range(2):
                nc.tensor.matmul(ps[:], oq[g][:, :, t * 128:(t + 1) * 128],
                                 wp8[g][:], start=(g == 0), stop=(g == 1),
                                 perf_mode=DR)
            nc.vector.scalar_tensor_tensor(
                out=x2[t][:], in0=ps[:], scalar=IWS, in1=xb[t][:],
                op0=ALU.mult, op1=ALU.add)
            ln_stats(x2[t], mvs2, t)

        def epilogue_ln2(t):
            h2 = hpool.tile([128, D], BF16, tag="h2", name="h2")
            ln_apply(x2[t], h2, mvs2, rst2, t, nc.gpsimd)
            transpose_pairs(h2, h2T8, t * 128, 128, CFG["h2t"])

        # ---------------- FFN per 256-token own chunk ----------------
        def ffn2_tile(f8c, tt, t):
            ps = psS.tile([128, 512], FP32, tag="sps", name="sps")
            for k in range(8):
                nc.tensor.matmul(
                    ps[:],
                    f8c[:, 2 * k:2 * k + 2, tt * 128:(tt + 1) * 128],
                    w28[k][:], start=(k == 0), stop=(k == 7), perf_mode=DR)
            o_t = opool.tile([128, D], FP32, tag="ot", name="ot")
            nc.vector.scalar_tensor_tensor(
                out=o_t[:], in0=ps[:], scalar=1.0 / (WS * WS), in1=x2[t][:],
                op0=ALU.mult, op1=ALU.add)
            nc.gpsimd.tensor_add(o_t[:], o_t[:], b2_b[:])
            nc.sync.dma_start(out_dram[t * 128:(t + 1) * 128, :], o_t[:])

        def ffn1_quarter(c, f8c, qi):
            for ht in range(4 * qi, 4 * qi + 4):
                ps = psW.tile([128, 256], FP32, tag="w", name="w")
                for k in range(2):
                    nc.tensor.matmul(
                        ps[:], w18[k][:, :, ht * 128:(ht + 1) * 128],
                        h2T8[k][:, :, c * 256:(c + 1) * 256],
                        start=(k == 0), stop=(k == 1), perf_mode=DR)
                if c == 3:
                    nc.scalar.activation(out=f8c[:, ht, :], in_=ps[:],
                                         func=ACT.Relu, bias=b1s[:, ht:ht + 1],
                                         scale=1.0)
                else:
                    nc.vector.tensor_scalar(
                        out=f8c[:, ht, :], in0=ps[:],
                        scalar1=b1s[:, ht:ht + 1], scalar2=0.0,
                        op0=ALU.add, op1=ALU.max)

        def ffn_pieces(c):
            """ffn chunk c as a list of small weave-able emitters."""
            box = {}

            def mk_f8c():
                box["f"] = f18pool.tile([128, 16, 256], FP8, tag="f8", name="f8")

            pieces = [mk_f8c]
            for qi in range(4):
                pieces.append(lambda qi=qi: ffn1_quarter(c, box["f"], qi))
            for tt in range(2):
                pieces.append(lambda tt=tt: ffn2_tile(box["f"], tt, 2 * c + tt))
            return pieces

        def ffn_chunk(c):
            for p in ffn_pieces(c):
                p()

        # ---------------- emission schedule ----------------
        def stats_group(g):
            for t in range(4 * g, 4 * g + 4):
                ln_stats(x_t[t], mvs1, t)
            ln_rstd(mvs1, rst1, 4 * g, 4)

        def kq(g, pr):
            return lambda: (k_group(g, (pr,)), q_slot(g, (pr,)))

        def _full_schedule():
            stats_group(0)
            for t in range(4):
                ln_tile(t)
            k_group(0, (0, 1))
            q_slot(0, (0, 1))
            att_head_pair(0, 0, [lambda: stats_group(1), lambda: ln_tile(4)])
            att_head_pair(0, 2, [lambda: ln_tile(5), kq(0, 2)])
            att_head_pair(0, 4, [lambda: ln_tile(6), lambda: ln_tile(7),
                                 kq(0, 3)])
            att_head_pair(0, 6, [kq(1, 0), kq(1, 1)])
            att_head_pair(1, 0, [lambda: stats_group(2), lambda: ln_tile(8),
                                 kq(1, 2)])
            att_head_pair(1, 2, [lambda: ln_tile(9), kq(1, 3)])
            att_head_pair(1, 4, [lambda: ln_tile(10), lambda: ln_tile(11),
                                 kq(2, 0)])
            att_head_pair(1, 6, [kq(2, 1), lambda: stats_group(3), kq(2, 2)])
            att_head_pair(2, 0, [lambda: ln_tile(12), kq(2, 3),
                                 lambda: ln_tile(13)])
            att_head_pair(2, 2, [lambda: ln_tile(14), lambda: ln_tile(15),
                                 lambda: epilogue_proj(0)])
            att_head_pair(2, 4, [lambda: epilogue_proj(1),
                                 lambda: ln_rstd(mvs2, rst2, 0, 2),
                                 lambda: epilogue_ln2(0), kq(3, 0)])
            att_head_pair(2, 6, [lambda: epilogue_ln2(1), kq(3, 1),
                                 kq(3, 2), lambda: epilogue_proj(2)])
            f0 = ffn_pieces(0)
            att_head_pair(3, 0, [kq(3, 3), lambda: epilogue_proj(3),
                                 lambda: ln_rstd(mvs2, rst2, 2, 2),
                                 lambda: epilogue_ln2(2),
                                 lambda: epilogue_ln2(3),
                                 f0[0], f0[1], f0[2]])
            f1p = ffn_pieces(1)
            att_head_pair(3, 2, [f0[3], f0[4], f0[5], f0[6],
                                 lambda: epilogue_proj(4),
                                 lambda: (epilogue_proj(5),
                                          ln_rstd(mvs2, rst2, 4, 2)),
                                 f1p[0], f1p[1]])
            f2 = ffn_pieces(2)
            att_head_pair(3, 4, [f1p[2], f1p[3], f1p[4], f1p[5], f1p[6],
                                 lambda: epilogue_ln2(4),
                                 lambda: epilogue_ln2(5), f2[0]])
            att_head_pair(3, 6, [f2[1], f2[2], f2[3], f2[4], f2[5], f2[6]])
            # tail (epilogues 6/7 read oq written by pair(3,6)'s tail)
            epilogue_proj(6)
            epilogue_proj(7)
            ln_rstd(mvs2, rst2, 6, 2)
            epilogue_ln2(6)
            epilogue_ln2(7)
            ffn_chunk(3)

        STAGE = int(_os.environ.get("STAGE", "8"))
        if STAGE == 12:
            # stage 8 + FFN chunks 0/1 at the proven interleave point
            for g in range(4):
                stats_group(g)
            for t in range(16):
                ln_tile(t)
            for g in range(4):
                k_group(g)
                q_slot(g)
            for s in range(3):
                for hp in range(4):
                    att_head_pair(s, 2 * hp)
            for t in range(4):
                epilogue_proj(t)
                if t % 2 == 1:
                    ln_rstd(mvs2, rst2, t - 1, 2)
            for t in range(4):
                epilogue_ln2(t)
            ffn_chunk(0)
            ffn_chunk(1)
            for hp in range(4):
                att_head_pair(3, 2 * hp)
            for t in range(4, OWN_NT):
                epilogue_proj(t)
                if t % 2 == 1:
                    ln_rstd(mvs2, rst2, t - 1, 2)
            for t in range(4, OWN_NT):
                epilogue_ln2(t)
            ffn_chunk(2)
            ffn_chunk(3)
        elif STAGE == 13:
            # stage 8 + only ffn chunk 0 at the interleave point
            for g in range(4):
                stats_group(g)
            for t in range(16):
                ln_tile(t)
            for g in range(4):
                k_group(g)
                q_slot(g)
            for s in range(3):
                for hp in range(4):
                    att_head_pair(s, 2 * hp)
            for t in range(4):
                epilogue_proj(t)
                if t % 2 == 1:
                    ln_rstd(mvs2, rst2, t - 1, 2)
            for t in range(4):
                epilogue_ln2(t)
            ffn_chunk(0)
            for hp in range(4):
                att_head_pair(3, 2 * hp)
            ffn_chunk(1)
            for t in range(4, OWN_NT):
                epilogue_proj(t)
                if t % 2 == 1:
                    ln_rstd(mvs2, rst2, t - 1, 2)
            for t in range(4, OWN_NT):
                epilogue_ln2(t)
            ffn_chunk(2)
            ffn_chunk(3)
        elif STAGE == 11:
            # between-slot interleave points only (no within-slot emissions)
            for g in range(4):
                stats_group(g)
            for t in range(16):
                ln_tile(t)
            for g in range(4):
                k_group(g)
                q_slot(g)
            for hp in range(4):
                att_head_pair(0, 2 * hp)
            for hp in range(4):
                att_head_pair(1, 2 * hp)
            epilogue_proj(0); epilogue_proj(1); ln_rstd(mvs2, rst2, 0, 2)
            epilogue_ln2(0); epilogue_ln2(1)
            for hp in range(4):
                att_head_pair(2, 2 * hp)
            epilogue_proj(2); epilogue_proj(3); ln_rstd(mvs2, rst2, 2, 2)
            epilogue_ln2(2); epilogue_ln2(3)
            ffn_chunk(0)
            for hp in range(4):
                att_head_pair(3, 2 * hp)
            epilogue_proj(4); epilogue_proj(5); ln_rstd(mvs2, rst2, 4, 2)
            epilogue_ln2(4); epilogue_ln2(5)
            ffn_chunk(1)
            epilogue_proj(6); epilogue_proj(7); ln_rstd(mvs2, rst2, 6, 2)
            epilogue_ln2(6); epilogue_ln2(7)
            ffn_chunk(2)
            ffn_chunk(3)
        elif STAGE == 10:
            # QKV upfront; epilogue/FFN interleaved after each slot and
            # between slot-3 pairs
            for g in range(4):
                stats_group(g)
            for t in range(16):
                ln_tile(t)
            for g in range(4):
                k_group(g)
                q_slot(g)
            for hp in range(4):
                att_head_pair(0, 2 * hp)
            for hp in range(4):
                att_head_pair(1, 2 * hp)
            epilogue_proj(0); epilogue_proj(1); ln_rstd(mvs2, rst2, 0, 2)
            epilogue_ln2(0); epilogue_ln2(1)
            att_head_pair(2, 0)
            att_head_pair(2, 2)
            epilogue_proj(2); epilogue_proj(3); ln_rstd(mvs2, rst2, 2, 2)
            att_head_pair(2, 4)
            epilogue_ln2(2); epilogue_ln2(3)
            att_head_pair(2, 6)
            ffn_chunk(0)
            att_head_pair(3, 0)
            epilogue_proj(4); epilogue_proj(5); ln_rstd(mvs2, rst2, 4, 2)
            att_head_pair(3, 2)
            epilogue_ln2(4); epilogue_ln2(5)
            att_head_pair(3, 4)
            ffn_chunk(1)
            att_head_pair(3, 6)
            ffn_chunk(2)
            epilogue_proj(6); epilogue_proj(7); ln_rstd(mvs2, rst2, 6, 2)
            epilogue_ln2(6); epilogue_ln2(7)
            ffn_chunk(3)
        elif STAGE == 8:
            # stage 4 plus ONE interleave point: slot 0-1 epilogues before slot3
            for g in range(4):
                stats_group(g)
            for t in range(16):
                ln_tile(t)
            for g in range(4):
                k_group(g)
                q_slot(g)
            for s in range(3):
                for hp in range(4):
                    att_head_pair(s, 2 * hp)
            for t in range(4):
                epilogue_proj(t)
                if t % 2 == 1:
                    ln_rstd(mvs2, rst2, t - 1, 2)
            for t in range(4):
                epilogue_ln2(t)
            for hp in range(4):
                att_head_pair(3, 2 * hp)
            for t in range(4, OWN_NT):
                epilogue_proj(t)
                if t % 2 == 1:
                    ln_rstd(mvs2, rst2, t - 1, 2)
            for t in range(4, OWN_NT):
                epilogue_ln2(t)
            for c in range(4):
                ffn_chunk(c)
        elif STAGE == 7:
            # QKV fully upfront; epilogue/FFN woven intra-pair into slots 2-3
            for g in range(4):
                stats_group(g)
            for t in range(16):
                ln_tile(t)
            for g in range(4):
                k_group(g)
                q_slot(g)
            for s in range(2):
                for hp in range(4):
                    att_head_pair(s, 2 * hp)
            att_head_pair(2, 0)
            att_head_pair(2, 2)
            att_head_pair(2, 4, [lambda: epilogue_proj(0),
                                 lambda: epilogue_proj(1),
                                 lambda: ln_rstd(mvs2, rst2, 0, 2)])
            att_head_pair(2, 6, [lambda: epilogue_ln2(0),
                                 lambda: epilogue_ln2(1),
                                 lambda: epilogue_proj(2)])
            f0 = ffn_pieces(0)
            att_head_pair(3, 0, [lambda: (epilogue_proj(3),
                                          ln_rstd(mvs2, rst2, 2, 2)),
                                 lambda: epilogue_ln2(2),
                                 lambda: epilogue_ln2(3),
                                 f0[0], f0[1], f0[2]])
            f1p = ffn_pieces(1)
            att_head_pair(3, 2, [f0[3], f0[4], f0[5], f0[6],
                                 lambda: epilogue_proj(4),
                                 lambda: (epilogue_proj(5),
                                          ln_rstd(mvs2, rst2, 4, 2)),
                                 f1p[0], f1p[1]])
            f2 = ffn_pieces(2)
            att_head_pair(3, 4, [f1p[2], f1p[3], f1p[4], f1p[5], f1p[6],
                                 lambda: epilogue_ln2(4),
                                 lambda: epilogue_ln2(5), f2[0]])
            att_head_pair(3, 6, [f2[1], f2[2], f2[3], f2[4], f2[5], f2[6]])
            epilogue_proj(6)
            epilogue_proj(7)
            ln_rstd(mvs2, rst2, 6, 2)
            epilogue_ln2(6)
            epilogue_ln2(7)
            ffn_chunk(3)
        elif STAGE == 6:
            # between-pair interleaving (no intra-pair weaving)
            stats_group(0)
            for t in range(4):
                ln_tile(t)
            k_group(0, (0, 1))
            q_slot(0, (0, 1))
            att_head_pair(0, 0)
            stats_group(1); ln_tile(4)
            att_head_pair(0, 2)
            ln_tile(5); kq(0, 2)()
            att_head_pair(0, 4)
            ln_tile(6); ln_tile(7); kq(0, 3)()
            att_head_pair(0, 6)
            kq(1, 0)(); kq(1, 1)()
            att_head_pair(1, 0)
            stats_group(2); ln_tile(8); kq(1, 2)()
            att_head_pair(1, 2)
            ln_tile(9); kq(1, 3)()
            att_head_pair(1, 4)
            ln_tile(10); ln_tile(11); kq(2, 0)()
            att_head_pair(1, 6)
            kq(2, 1)(); stats_group(3); kq(2, 2)()
            att_head_pair(2, 0)
            ln_tile(12); ln_tile(13); kq(2, 3)()
            att_head_pair(2, 2)
            ln_tile(14); ln_tile(15)
            att_head_pair(2, 4)
            epilogue_proj(0); epilogue_proj(1); ln_rstd(mvs2, rst2, 0, 2)
            att_head_pair(2, 6)
            epilogue_ln2(0); epilogue_ln2(1); kq(3, 0)(); kq(3, 1)()
            att_head_pair(3, 0)
            kq(3, 2)(); kq(3, 3)()
            epilogue_proj(2); epilogue_proj(3); ln_rstd(mvs2, rst2, 2, 2)
            att_head_pair(3, 2)
            epilogue_ln2(2); epilogue_ln2(3)
            ffn_chunk(0)
            att_head_pair(3, 4)
            ffn_chunk(1)
            epilogue_proj(4); epilogue_proj(5); ln_rstd(mvs2, rst2, 4, 2)
            att_head_pair(3, 6)
            epilogue_ln2(4); epilogue_ln2(5)
            ffn_chunk(2)
            epilogue_proj(6); epilogue_proj(7); ln_rstd(mvs2, rst2, 6, 2)
            epilogue_ln2(6); epilogue_ln2(7)
            ffn_chunk(3)
        elif STAGE == 5:
            # full ln/kq weave; epilogues+FFN sequential
            stats_group(0)
            for t in range(4):
                ln_tile(t)
            k_group(0, (0, 1))
            q_slot(0, (0, 1))
            att_head_pair(0, 0, [lambda: stats_group(1), lambda: ln_tile(4)])
            att_head_pair(0, 2, [lambda: ln_tile(5), kq(0, 2)])
            att_head_pair(0, 4, [lambda: ln_tile(6), lambda: ln_tile(7),
                                 kq(0, 3)])
            att_head_pair(0, 6, [kq(1, 0), kq(1, 1)])
            att_head_pair(1, 0, [lambda: stats_group(2), lambda: ln_tile(8),
                                 kq(1, 2)])
            att_head_pair(1, 2, [lambda: ln_tile(9), kq(1, 3)])
            att_head_pair(1, 4, [lambda: ln_tile(10), lambda: ln_tile(11),
                                 kq(2, 0)])
            att_head_pair(1, 6, [kq(2, 1), lambda: stats_group(3), kq(2, 2)])
            att_head_pair(2, 0, [lambda: ln_tile(12), kq(2, 3),
                                 lambda: ln_tile(13)])
            att_head_pair(2, 2, [lambda: ln_tile(14), lambda: ln_tile(15)])
            att_head_pair(2, 4, [kq(3, 0), kq(3, 1)])
            att_head_pair(2, 6, [kq(3, 2), kq(3, 3)])
            for hp in range(4):
                att_head_pair(3, 2 * hp)
            for t in range(OWN_NT):
                epilogue_proj(t)
                if t % 2 == 1:
                    ln_rstd(mvs2, rst2, t - 1, 2)
            for t in range(OWN_NT):
                epilogue_ln2(t)
            for c in range(4):
                ffn_chunk(c)
        elif STAGE <= 4:
            # reduced program for HW bisection
            for g in range(4):
                stats_group(g)
            for t in range(16):
                ln_tile(t)
            for g in range(4):
                k_group(g)
                q_slot(g)
            if STAGE >= 2:
                for s in range(4):
                    for hp in range(4):
                        att_head_pair(s, 2 * hp)
            if STAGE >= 3:
                for t in range(OWN_NT):
                    epilogue_proj(t)
                    if t % 2 == 1:
                        ln_rstd(mvs2, rst2, t - 1, 2)
                for t in range(OWN_NT):
                    epilogue_ln2(t)
            if STAGE >= 4:
                for c in range(4):
                    ffn_chunk(c)
            else:
                for t in range(OWN_NT):
                    o_t = opool.tile([128, D], FP32, tag="ot", name="ot")
                    nc.vector.tensor_copy(
                        o_t[:], (x2 if STAGE >= 3 else xb)[t][:])
                    nc.sync.dma_start(out_dram[t * 128:(t + 1) * 128, :],
                                      o_t[:])
        else:
            _full_schedule()

    import os
    if not os.environ.get("SKIP_WAITFIX"):
        _split_multi_waits(nc)
    return nc


_NC_CACHE = None


def _get_nc():
    global _NC_CACHE
    if _NC_CACHE is None:
        _NC_CACHE = _build_program()
    return _NC_CACHE


# ---------------------------------------------------------------------------
# Host side
# ---------------------------------------------------------------------------
def _to_fp8(a):
    return np.clip(a, -240.0, 240.0).astype(F8NP)


def _pack_dr(W):
    """[512, N] -> [2, 128, 2, N] fp8 with W scaled by WS."""
    Ws = np.asarray(W, np.float64) * WS
    out = np.empty((2, 128, 2, Ws.shape[1]), np.float64)
    for kp in range(2):
        for j in range(2):
            out[kp, :, j, :] = Ws[kp * 256 + j * 128:kp * 256 + (j + 1) * 128, :]
    return _to_fp8(out)


def _fold_weights(Wq, bq, Wk, bk, Wv, bv, Wp, bp, W1, b1, W2, b2, g1, be1,
                  g2, be2):
    f64 = np.float64
    Wq_e = (g1.astype(f64)[None, :, None] * Wq.astype(f64))      # [H,D,DK]
    Wk_e = (g1.astype(f64)[None, :, None] * Wk.astype(f64))
    Wv_e = (g1.astype(f64)[None, :, None] * Wv.astype(f64))
    bq_e = bq.astype(f64) + np.einsum("d,hdk->hk", be1.astype(f64), Wq.astype(f64))
    bk_e = bk.astype(f64) + np.einsum("d,hdk->hk", be1.astype(f64), Wk.astype(f64))
    bv_e = bv.astype(f64) + np.einsum("d,hdk->hk", be1.astype(f64), Wv.astype(f64))
    W1_e = g2.astype(f64)[:, None] * W1.astype(f64)
    b1_e = b1.astype(f64) + be2.astype(f64) @ W1.astype(f64)

    def head_major(W):  # [H,D,DK] -> [D, H*DK]
        return np.transpose(W, (1, 0, 2)).reshape(D, H * DK)

    out = {}
    out["wq8"] = _pack_dr(head_major(Wq_e))
    out["wk8"] = _pack_dr(head_major(Wk_e))
    out["wv8"] = _pack_dr(head_major(Wv_e))
    # wp8: quad g covers heads 4g..4g+3; [128(2 heads x 64k), 2(head pair), D]
    Wp_f = Wp.astype(f64).reshape(H, DK, D)
    wp8 = np.empty((2, 128, 2, D), np.float64)
    for g in range(2):
        for j in range(2):
            h0 = 4 * g + 2 * j
            wp8[g, 0:64, j, :] = Wp_f[h0] * WS
            wp8[g, 64:128, j, :] = Wp_f[h0 + 1] * WS
    out["wp8"] = _to_fp8(wp8)
    out["w18"] = _pack_dr(W1_e)
    # w28: [8 hidpair, 128, 2, D]
    W2_f = W2.astype(f64) * WS
    w28 = np.empty((8, 128, 2, D), np.float64)
    for hp in range(8):
        for j in range(2):
            w28[hp, :, j, :] = W2_f[hp * 256 + j * 128:hp * 256 + (j + 1) * 128, :]
    out["w28"] = _to_fp8(w28)

    bqk = np.zeros((128, 8), np.float32)
    for pr in range(4):
        bqk[:, pr] = np.concatenate([bq_e[2 * pr], bq_e[2 * pr + 1]])
        bqk[:, 4 + pr] = np.concatenate([bk_e[2 * pr], bk_e[2 * pr + 1]])
    out["bqk"] = bqk
    out["b1s"] = np.ascontiguousarray(
        (WS * b1_e).reshape(16, 128).T).astype(np.float32)
    out["bv_row"] = bv_e.reshape(1, H * DK).astype(np.float32)
    out["bp_row"] = bp.reshape(1, D).astype(np.float32)
    out["b2_row"] = b2.reshape(1, D).astype(np.float32)
    return out


def _build_masks(p):
    """[4, 128, 2, 1024] fp8: per slot, DR rhs for the last quad's mask."""
    perm = PERM[p]
    masks = np.zeros((4, 128, 2, 1024), np.float32)
    for s in range(4):
        U = SPANS[s]
        qc = perm[OWN_POS[s]]              # original own chunk for this slot
        q_tok = qc * 256 + np.arange(256)  # original q token ids
        for j in range(4):
            u = (U - 4) + j                # x' kv tile index
            kv_tok = perm[u // 2] * 256 + (u % 2) * 128 + np.arange(128)
            m = np.where(kv_tok[:, None] <= q_tok[None, :], 0.0, NEG)
            masks[s, :, 0, j * 256:(j + 1) * 256] = m
    return masks.astype(F8NP)


def kernel(x, Wq, bq, Wk, bk, Wv, bv, Wp, bp, W1, b1, W2, b2, g1, be1, g2, be2):
    x = np.asarray(x, np.float32)
    folded = _fold_weights(
        np.asarray(Wq), np.asarray(bq), np.asarray(Wk), np.asarray(bk),
        np.asarray(Wv), np.asarray(bv), np.asarray(Wp), np.asarray(bp),
        np.asarray(W1), np.asarray(b1), np.asarray(W2), np.asarray(b2),
        np.asarray(g1), np.asarray(be1), np.asarray(g2), np.asarray(be2))

    masks_by_p = [_build_masks(0), _build_masks(1)]
    in_maps = []
    for c in range(8):
        b, p = c // 2, c % 2
        perm = PERM[p]
        xp = np.concatenate([x[b, pc * 256:(pc + 1) * 256] for pc in perm])
        m = dict(folded)
        m["xp"] = np.ascontiguousarray(xp)
        m["masks8"] = masks_by_p[p]
        in_maps.append(m)

    nc = _get_nc()
    res = run_bass_kernel_spmd(nc, in_maps, list(range(8)))

    out = np.empty((B, T, D), np.float32)
    for c in range(8):
        b, p = c // 2, c % 2
        perm = PERM[p]
        o = res.results[c]["out"]
        for s in range(4):
            oc = perm[OWN_POS[s]]
            out[b, oc * 256:(oc + 1) * 256] = o[s * 256:(s + 1) * 256]
    return out


# revision 5
# speedup vs baseline: 1.0061x; 1.0061x over previous
"""Trainium2 Bass kernel for a dense transformer block (nn_Block_83880711291003).

Full (unsharded) inputs in, full output out. 8 NeuronCores:
  core c -> batch b = c//2, parity p = c%2.
Each core computes LN1 + K/V over its batch's full 2048 tokens and owns 1024
query tokens (4 chunks of 256). A host-side chunk permutation places each
parity's own chunks at fixed x'-positions {1,3,5,7} so ONE SPMD program works
for both parities: attention slot s (span 4/8/12/16 kv-tiles over the x'
prefix) handles the own chunk at x' position 2s+1; causality is enforced by
host-built additive masks folded into the score PSUM via DoubleRow identity
matmuls. Heavy matmuls (QKV, AV, out-proj, FFN) run as fp8e4m3 DoubleRow
(256-deep contraction, 0.5 cycles/col); scores stay bf16.
"""

import sys

for _p in ("/opt/trn_rl_repo", "/root/.axon_site/_ro/trn_rl_repo"):
    if _p not in sys.path:
        sys.path.append(_p)

from contextlib import ExitStack

import ml_dtypes
import numpy as np

import concourse.bass as bass
import concourse.tile as tile
from concourse import mybir
from concourse.bass_utils import run_bass_kernel_spmd
from concourse.masks import make_identity
from concourse.vector_clock import ScopedClock

FP32 = mybir.dt.float32
BF16 = mybir.dt.bfloat16
FP8 = mybir.dt.float8e4
BFNP = ml_dtypes.bfloat16
F8NP = ml_dtypes.float8_e4m3

ACT = mybir.ActivationFunctionType
ALU = mybir.AluOpType
DR = mybir.MatmulPerfMode.DoubleRow

B, T, D = 4, 2048, 512
H, DK = 8, 64
FF = 4 * D
EPS = 1e-5
WS = 16.0          # fp8 weight scale
IWS = 1.0 / WS
NEG = -240.0       # additive mask value (fp8e4m3 max normal magnitude)
SPANS = (4, 8, 12, 16)      # kv span per slot, in 128-tiles
OWN_POS = (1, 3, 5, 7)      # x' chunk position of each slot's own q chunk
# per-parity x' chunk order: x' position i holds original chunk PERM[p][i]
PERM = ((1, 0, 2, 3, 5, 4, 6, 7), (0, 1, 3, 2, 4, 5, 7, 6))
OWN_T = 1024
OWN_NT = 8

# ---------------------------------------------------------------------------
# Workaround: this walrus build rejects >1 semaphore wait per instruction.
# ---------------------------------------------------------------------------
_uid = [0]


def _split_multi_waits(nc):
    for blk in nc.m.functions[0].blocks:
        insts = list(blk.instructions)
        out, changed = [], False
        for inst in insts:
            si = inst.sync_info
            waits = list(si.on_wait) if si else []
            if len(waits) > 1:
                changed = True
                for w in waits[:-1]:
                    _uid[0] += 1
                    nop = mybir.InstNoOp(name=f"I-waitfix-{_uid[0]}", ins=[], outs=[])
                    nop.engine = inst.engine
                    nop.sync_info = mybir.SyncInfo(on_wait=[w], on_update=[])
                    out.append(nop)
                inst.sync_info = mybir.SyncInfo(
                    on_wait=[waits[-1]], on_update=list(si.on_update)
                )
            out.append(inst)
        if changed:
            blk.instructions = out


def _patched_drain_and_barrier(self, tick_clock, wait_clock):
    nc = self.nc
    probe = nc.sync.nop()
    wait_clock.add_sem_waits(probe.ins, ScopedClock({None: tick_clock.global_clock}))
    nc.sync.drain()
    nc.all_engine_barrier()
    popped = nc._tile_sem_poison_stack.pop()
    assert popped is self._sem_poison
    nc.clear_and_free_semaphores(list(self.sems.allocated().values()))
    nc.all_engine_barrier()


tile.TileContext._drain_and_barrier = _patched_drain_and_barrier


# ---------------------------------------------------------------------------
# Device program (identical on all 8 cores)
# ---------------------------------------------------------------------------
import os as _os
CFG = {
    "ht_early": _os.environ.get("CFG_HT_EARLY", "act"),   # hT8 copies t<4
    "ht_mid": _os.environ.get("CFG_HT_MID", "act"),       # hT8 copies t>=4
    "h2t": _os.environ.get("CFG_H2T", "dve"),             # h2T8 copies
    "oq_late_act": _os.environ.get("CFG_OQ", "0") == "1", # oq s>=2 on ACT
    "kq0_act": _os.environ.get("CFG_KQ0", "1") == "1",    # kq g0/s0 on ACT
}


def _build_program():
    nc = bass.Bass("TRN2", target_bir_lowering=False, debug=False)

    din = {}
    for name, shape, dt in [
        ("xp", [T, D], FP32),                 # permuted x for this core
        ("wq8", [2, 128, 2, D], FP8),
        ("wk8", [2, 128, 2, D], FP8),
        ("wv8", [2, 128, 2, D], FP8),
        ("wp8", [2, 128, 2, D], FP8),         # [quad A/B, p, j, d]
        ("w18", [2, 128, 2, FF], FP8),
        ("w28", [8, 128, 2, D], FP8),
        ("bqk", [128, 8], FP32),              # cols 0-3 bq per pr; 4-7 bk
        ("b1s", [128, 16], FP32),             # WS * b1 per hidden tile
        ("bv_row", [1, D], FP32),
        ("bp_row", [1, D], FP32),
        ("b2_row", [1, D], FP32),
        ("masks8", [4, 128, 2, 1024], FP8),   # per slot: DR mask rhs (j1=0)
    ]:
        din[name] = nc.dram_tensor(name, shape, dt, kind="ExternalInput").ap()
    out_dram = nc.dram_tensor("out", [OWN_T, D], FP32, kind="ExternalOutput").ap()

    with tile.TileContext(nc) as tc, ExitStack() as ctx:
        P = ctx.enter_context

        wpool = P(tc.tile_pool(name="weights", bufs=1))
        persist = P(tc.tile_pool(name="persist", bufs=1))
        xio = P(tc.tile_pool(name="xio", bufs=5))
        small = P(tc.tile_pool(name="small", bufs=6))
        hpool = P(tc.tile_pool(name="htok", bufs=4))
        ppool = P(tc.tile_pool(name="pT", bufs=4))
        opool = P(tc.tile_pool(name="outio", bufs=3))
        f18pool = P(tc.tile_pool(name="f18", bufs=2))
        psS = P(tc.tile_pool(name="psS", bufs=2, space="PSUM"))   # [128,1024]
        psA = P(tc.tile_pool(name="psA", bufs=2, space="PSUM"))   # AV accum
        psW = P(tc.tile_pool(name="psW", bufs=2, space="PSUM"))   # [128,512]

        # ---- input DMAs: x tiles interleaved with weights (SP queue);
        # masks/broadcasts/FFN weights via gpsimd (SWDGE) ----
        x_t = [None] * 16
        own_slot = {2: 0, 3: 1, 6: 2, 7: 3, 10: 4, 11: 5, 14: 6, 15: 7}
        wq8 = [wpool.tile([128, 2, D], FP8, tag=f"wq8{k}", name=f"wq8{k}")
               for k in range(2)]
        wk8 = [wpool.tile([128, 2, D], FP8, tag=f"wk8{k}", name=f"wk8{k}")
               for k in range(2)]
        wv8 = [wpool.tile([128, 2, D], FP8, tag=f"wv8{k}", name=f"wv8{k}")
               for k in range(2)]
        wp8 = [wpool.tile([128, 2, D], FP8, tag=f"wp8{k}", name=f"wp8{k}")
               for k in range(2)]

        def dma_x(t):
            if t in own_slot:
                x_t[t] = persist.tile([128, D], FP32, tag=f"xo{t}", name=f"xo{t}")
            else:
                x_t[t] = xio.tile([128, D], FP32, tag="xin", name="xin")
            nc.sync.dma_start(x_t[t][:], din["xp"][t * 128:(t + 1) * 128, :])

        bqk = wpool.tile([128, 8], FP32, tag="bqk", name="bqk")
        b1s = wpool.tile([128, 16], FP32, tag="b1s", name="b1s")
        for t in range(4):
            dma_x(t)
        for k in range(2):
            nc.sync.dma_start(wv8[k][:], din["wv8"][k])
            nc.sync.dma_start(wk8[k][:], din["wk8"][k])
        for t in range(4, 8):
            dma_x(t)
        for k in range(2):
            nc.sync.dma_start(wq8[k][:], din["wq8"][k])
        nc.sync.dma_start(bqk[:], din["bqk"][:])
        for t in range(8, 16):
            dma_x(t)
        for k in range(2):
            nc.sync.dma_start(wp8[k][:], din["wp8"][k])
        nc.sync.dma_start(b1s[:], din["b1s"][:])

        # constants first: they gate the first transposes/LN ops and must not
        # queue behind SWDGE descriptor generation on the Pool engine
        ident8 = wpool.tile([128, 128], FP8, tag="ident8", name="ident8")
        make_identity(nc, ident8[:])
        identB = wpool.tile([128, 128], BF16, tag="identB", name="identB")
        make_identity(nc, identB[:])
        identDR = wpool.tile([128, 2, 128], FP8, tag="identDR", name="identDR")
        nc.vector.memset(identDR[:, 1, :], 0.0)
        make_identity(nc, identDR[:, 0, :])
        eps_t = wpool.tile([128, 1], FP32, tag="eps", name="eps")
        nc.vector.memset(eps_t[:], EPS)
        # preload ACT tables for Exp/Relu/Sqrt before the hot loops
        warm = wpool.tile([128, 1], FP32, tag="warm", name="warm")
        nc.scalar.activation(out=warm[:], in_=eps_t[:], func=ACT.Exp, scale=1.0)
        nc.scalar.activation(out=warm[:], in_=eps_t[:], func=ACT.Relu, scale=1.0)
        nc.scalar.activation(out=warm[:], in_=eps_t[:], func=ACT.Sqrt, scale=1.0)

        def bcast_row(name):
            t = wpool.tile([128, D], FP32, tag=f"bc_{name}", name=f"bc_{name}")
            src = din[name]
            ap = bass.AP(tensor=src.tensor, offset=src.offset,
                         ap=[[0, 128], src.ap[1]])
            nc.scalar.dma_start(out=t[:], in_=ap)
            return t

        bv_b = bcast_row("bv_row")
        bp_b = bcast_row("bp_row")
        b2_b = bcast_row("b2_row")

        masks8 = wpool.tile([128, 4, 2, 1024], FP8, tag="masks8", name="masks8")
        nc.sync.dma_start(masks8[:], din["masks8"].rearrange("s p j f -> p s j f"))
        w18 = [wpool.tile([128, 2, FF], FP8, tag=f"w18{k}", name=f"w18{k}")
               for k in range(2)]
        w28 = [wpool.tile([128, 2, D], FP8, tag=f"w28{k}", name=f"w28{k}")
               for k in range(8)]
        for k in range(2):
            nc.sync.dma_start(w18[k][:], din["w18"][k])
        for k in range(8):
            nc.sync.dma_start(w28[k][:], din["w28"][k])

        # ---- long-lived activations ----
        hT8 = [persist.tile([128, 2, T], FP8, tag=f"hT8{k}", name=f"hT8{k}")
               for k in range(2)]
        kT = [persist.tile([128, T], BF16, tag=f"kT{pr}", name=f"kT{pr}")
              for pr in range(4)]
        qT = [persist.tile([128, 4, 256], BF16, tag=f"qT{pr}", name=f"qT{pr}")
              for pr in range(4)]
        # v8[p, kvpair, j, h, k]; col 64 is the ones column (denominator)
        v8 = persist.tile([128, 8, 2, H, 65], FP8, tag="v8", name="v8")
        nc.vector.memset(v8[:, :, :, :, 64], 1.0)
        oq = [persist.tile([128, 2, OWN_T], FP8, tag=f"oq{g}", name=f"oq{g}")
              for g in range(2)]
        xb = [persist.tile([128, D], FP32, tag=f"xb{t}", name=f"xb{t}")
              for t in range(OWN_NT)]
        x2 = [persist.tile([128, D], FP32, tag=f"x2_{t}", name=f"x2_{t}")
              for t in range(OWN_NT)]
        h2T8 = [persist.tile([128, 2, OWN_T], FP8, tag=f"h2T8{k}", name=f"h2T8{k}")
                for k in range(2)]

        # LN statistics, batched: per-tile bn stats land in a shared [128,2,N]
        # tile; one Sqrt + one reciprocal per tile-group keeps ACT's queue
        # clear of per-tile LN work.
        mvs1 = persist.tile([128, 2, 16], FP32, tag="mvs1", name="mvs1")
        rst1 = persist.tile([128, 16], FP32, tag="rst1", name="rst1")
        mvs2 = persist.tile([128, 2, 8], FP32, tag="mvs2", name="mvs2")
        rst2 = persist.tile([128, 8], FP32, tag="rst2", name="rst2")

        def ln_stats(x_in, mvs, t):
            stats = small.tile([128, 6], FP32, tag="bnst", name="bnst")
            nc.vector.bn_stats(out=stats[:], in_=x_in[:])
            mv = small.tile([128, 2], FP32, tag="bnmv", name="bnmv")
            nc.vector.bn_aggr(out=mv[:], in_=stats[:])
            nc.vector.tensor_copy(mvs[:, :, t], mv[:])

        def ln_rstd(mvs, rst, t0, n):
            rs = small.tile([128, 4], FP32, tag="rs", name="rs")
            nc.scalar.activation(out=rs[:, 0:n], in_=mvs[:, 1, t0:t0 + n],
                                 func=ACT.Sqrt, bias=eps_t[:], scale=1.0)
            nc.vector.reciprocal(out=rst[:, t0:t0 + n], in_=rs[:, 0:n])

        def ln_apply(x_in, h_out, mvs, rst, t, eng=None):
            eng = eng or nc.vector
            eng.tensor_scalar(
                out=h_out[:], in0=x_in[:], scalar1=mvs[:, 0, t:t + 1],
                scalar2=rst[:, t:t + 1], op0=ALU.subtract, op1=ALU.mult)

        def transpose_pairs(h_t, dst, dst_col, n, copy_eng="mix"):
            """dst[k][:, j, dst_col:+n] = h_t[:, (2k+j)*128:+128].T
            (bf16 transpose through PSUM, cast to fp8 on the copy out)."""
            for k in range(2):
                ps = psW.tile([128, 2, 128], BF16, tag="w", name="w")
                for j in range(2):
                    nc.tensor.matmul(ps[:, j, :],
                                     h_t[:, (2 * k + j) * 128:(2 * k + j + 1) * 128],
                                     identB[:], is_transpose=True,
                                     start=(j == 0), stop=(j == 1),
                                     skip_group_check=True)
                if copy_eng == "act" or (copy_eng == "mix" and k == 1):
                    nc.scalar.copy(dst[k][:, :, dst_col:dst_col + n], ps[:])
                else:
                    nc.vector.tensor_copy(dst[k][:, :, dst_col:dst_col + n],
                                          ps[:])

        # ---------------- stage A: LN1 + QKV per tile/group ----------------
        def ln_tile(t):
            h_t = hpool.tile([128, D], BF16, tag="h1", name="h1")
            ln_apply(x_t[t], h_t, mvs1, rst1, t,
                     nc.gpsimd if t >= 4 else nc.vector)
            transpose_pairs(h_t, hT8, t * 128, 128,
                            CFG["ht_early"] if t < 4 else CFG["ht_mid"])
            if t in own_slot:
                nc.gpsimd.tensor_add(xb[own_slot[t]][:], x_t[t][:], bp_b[:])
            # V projection for this tile
            ps = psW.tile([128, D], FP32, tag="w", name="w")
            for k in range(2):
                nc.tensor.matmul(ps[:], hT8[k][:, :, t * 128:(t + 1) * 128],
                                 wv8[k][:], start=(k == 0), stop=(k == 1),
                                 perf_mode=DR)
            nc.vector.scalar_tensor_tensor(
                out=v8[:, t // 2, t % 2, :, 0:64],
                in0=ps[:].rearrange("p (h k) -> p h k", h=H),
                scalar=IWS,
                in1=bv_b[:].rearrange("p (h k) -> p h k", h=H),
                op0=ALU.mult, op1=ALU.add)

        def k_group(g, prs=range(4)):
            """kT for 512-token group g (x' tiles 4g..4g+3)."""
            for pr in prs:
                ps = psW.tile([128, D], FP32, tag="w", name="w")
                for k in range(2):
                    nc.tensor.matmul(
                        ps[:], wk8[k][:, :, pr * 128:(pr + 1) * 128],
                        hT8[k][:, :, g * 512:(g + 1) * 512],
                        start=(k == 0), stop=(k == 1), perf_mode=DR)
                if g == 0 and CFG["kq0_act"]:
                    nc.scalar.activation(
                        out=kT[pr][:, g * 512:(g + 1) * 512], in_=ps[:],
                        func=ACT.Identity, bias=bqk[:, 4 + pr:5 + pr],
                        scale=IWS)
                else:
                    nc.vector.tensor_scalar(
                        out=kT[pr][:, g * 512:(g + 1) * 512], in0=ps[:],
                        scalar1=IWS, scalar2=bqk[:, 4 + pr:5 + pr],
                        op0=ALU.mult, op1=ALU.add)

        def q_slot(s, prs=range(4)):
            pos = OWN_POS[s]
            for pr in prs:
                ps = psW.tile([128, 256], FP32, tag="w", name="w")
                for k in range(2):
                    nc.tensor.matmul(
                        ps[:], wq8[k][:, :, pr * 128:(pr + 1) * 128],
                        hT8[k][:, :, pos * 256:(pos + 1) * 256],
                        start=(k == 0), stop=(k == 1), perf_mode=DR)
                if s == 0 and CFG["kq0_act"]:
                    nc.scalar.activation(
                        out=qT[pr][:, s, :], in_=ps[:], func=ACT.Identity,
                        bias=bqk[:, pr:pr + 1], scale=IWS)
                else:
                    nc.vector.tensor_scalar(
                        out=qT[pr][:, s, :], in0=ps[:],
                        scalar1=IWS, scalar2=bqk[:, pr:pr + 1],
                        op0=ALU.mult, op1=ALU.add)

        # ---------------- attention (software-pipelined head pair) --------
        def att_scores(s, h, q):
            """scores + mask + exp for quad q of (slot s, head h) -> pt8."""
            nquads = SPANS[s] // 4
            pr, sub = h // 2, h % 2
            krows = kT[pr][sub * 64:(sub + 1) * 64, :]
            qrows = qT[pr][sub * 64:(sub + 1) * 64, s, :]
            sps = psS.tile([128, 4, 256], FP32, tag="sps", name="sps")
            masked = (q == nquads - 1)
            for j in range(4):
                u = q * 4 + j
                nc.tensor.matmul(sps[:, j, :],
                                 krows[:, u * 128:(u + 1) * 128], qrows,
                                 start=(j % 2 == 0),
                                 stop=(j % 2 == 1 and not masked),
                                 skip_group_check=True)
            if masked:
                for half in range(2):
                    nc.tensor.matmul(
                        sps[:, 2 * half:2 * half + 2, :].rearrange(
                            "p j f -> p (j f)"),
                        identDR[:],
                        masks8[:, s, :, half * 512:(half + 1) * 512],
                        start=False, stop=True, perf_mode=DR,
                        skip_group_check=True)
            pt8 = ppool.tile([128, 4, 256], FP8, tag="pt8", name="pt8")
            nc.scalar.activation(out=pt8[:], in_=sps[:], func=ACT.Exp,
                                 scale=0.125)
            return pt8

        def att_av(s, h, q, pt8, ops, oi):
            nquads = SPANS[s] // 4
            for jp in range(2):
                kvp = q * 2 + jp
                for sb in range(2):
                    nc.tensor.matmul(
                        ops[:, 2 * oi + sb, :],
                        pt8[:, 2 * jp:2 * jp + 2, sb * 128:(sb + 1) * 128],
                        v8[:, kvp, :, h, :],
                        start=(q == 0 and jp == 0 and oi == 0 and sb == 0),
                        stop=(q == nquads - 1 and jp == 1),
                        perf_mode=DR, skip_group_check=True)

        def att_head_pair(s, h0, weave=()):
            """heads h0, h0+1 with scores/AV software pipelining; `weave` is a
            list of small zero-arg emitters drained at stage boundaries."""
            nquads = SPANS[s] // 4
            weave = list(weave)
            ops = psA.tile([128, 4, 65], FP32, tag="ops", name="ops")
            pend = []
            for q in range(nquads):
                pt_a = att_scores(s, h0, q)
                pend.append((h0, q, pt_a, 0))
                if len(pend) > 1:
                    hh, qq, pt, oi = pend.pop(0)
                    att_av(s, hh, qq, pt, ops, oi)
                if weave:
                    weave.pop(0)()
                pt_b = att_scores(s, h0 + 1, q)
                pend.append((h0 + 1, q, pt_b, 1))
                hh, qq, pt, oi = pend.pop(0)
                att_av(s, hh, qq, pt, ops, oi)
                if weave:
                    weave.pop(0)()
            hh, qq, pt, oi = pend.pop(0)
            att_av(s, hh, qq, pt, ops, oi)
            for fn in weave:
                fn()
            rc = small.tile([128, 4], FP32, tag="rc", name="rc")
            nc.vector.reciprocal(out=rc[:], in_=ops[:, :, 64])
            for sb in range(2):
                tcol = (s * 2 + sb) * 128
                ps = psW.tile([128, 128], BF16, tag="w", name="w")
                for i in range(2):
                    oS = small.tile([128, 64], BF16, tag="oS", name="oS")
                    nc.vector.tensor_scalar(
                        out=oS[:], in0=ops[:, 2 * i + sb, 0:64],
                        scalar1=rc[:, 2 * i + sb:2 * i + sb + 1],
                        scalar2=None, op0=ALU.mult)
                    nc.tensor.transpose(ps[i * 64:(i + 1) * 64, :], oS[:],
                                        identB[:])
                if (s >= 2 and CFG["oq_late_act"]) or sb == 1:
                    nc.scalar.copy(
                        oq[h0 // 4][:, (h0 % 4) // 2, tcol:tcol + 128], ps[:])
                else:
                    nc.vector.tensor_copy(
                        oq[h0 // 4][:, (h0 % 4) // 2, tcol:tcol + 128], ps[:])

        # ---------------- per-own-tile epilogue: proj + LN2 ----------------
        def epilogue_proj(t):
            ps = psW.tile([128, D], FP32, tag="w", name="w")
            for g in range(2):
                nc.tensor.matmul(ps[:], oq[g][:, :, t * 128:(t + 1) * 128],
                                 wp8[g][:], start=(g == 0), stop=(g == 1),
                                 perf_mode=DR)
            nc.vector.scalar_tensor_tensor(
                out=x2[t][:], in0=ps[:], scalar=IWS, in1=xb[t][:],
                op0=ALU.mult, op1=ALU.add)
            ln_stats(x2[t], mvs2, t)

        def epilogue_ln2(t):
            h2 = hpool.tile([128, D], BF16, tag="h2", name="h2")
            ln_apply(x2[t], h2, mvs2, rst2, t, nc.gpsimd)
            transpose_pairs(h2, h2T8, t * 128, 128, CFG["h2t"])

        # ---------------- FFN per 256-token own chunk ----------------
        def ffn2_tile(f8c, tt, t):
            ps = psS.tile([128, 512], FP32, tag="sps", name="sps")
            for k in range(8):
                nc.tensor.matmul(
                    ps[:],
                    f8c[:, 2 * k:2 * k + 2, tt * 128:(tt + 1) * 128],
                    w28[k][:], start=(k == 0), stop=(k == 7), perf_mode=DR)
            o_t = opool.tile([128, D], FP32, tag="ot", name="ot")
            nc.vector.scalar_tensor_tensor(
                out=o_t[:], in0=ps[:], scalar=1.0 / (WS * WS), in1=x2[t][:],
                op0=ALU.mult, op1=ALU.add)
            nc.gpsimd.tensor_add(o_t[:], o_t[:], b2_b[:])
            nc.sync.dma_start(out_dram[t * 128:(t + 1) * 128, :], o_t[:])

        def ffn1_quarter(c, f8c, qi):
            for ht in range(4 * qi, 4 * qi + 4):
                ps = psW.tile([128, 256], FP32, tag="w", name="w")
                for k in range(2):
                    nc.tensor.matmul(
                        ps[:], w18[k][:, :, ht * 128:(ht + 1) * 128],
                        h2T8[k][:, :, c * 256:(c + 1) * 256],
                        start=(k == 0), stop=(k == 1), perf_mode=DR)
                if c >= int(_os.environ.get("CFG_RELU_ACT_FROM", "1")):
                    nc.scalar.activation(out=f8c[:, ht, :], in_=ps[:],
                                         func=ACT.Relu, bias=b1s[:, ht:ht + 1],
                                         scale=1.0)
                else:
                    nc.vector.tensor_scalar(
                        out=f8c[:, ht, :], in0=ps[:],
                        scalar1=b1s[:, ht:ht + 1], scalar2=0.0,
                        op0=ALU.add, op1=ALU.max)

        def ffn_pieces(c):
            """ffn chunk c as a list of small weave-able emitters."""
            box = {}

            def mk_f8c():
                box["f"] = f18pool.tile([128, 16, 256], FP8, tag="f8", name="f8")

            pieces = [mk_f8c]
            for qi in range(4):
                pieces.append(lambda qi=qi: ffn1_quarter(c, box["f"], qi))
            for tt in range(2):
                pieces.append(lambda tt=tt: ffn2_tile(box["f"], tt, 2 * c + tt))
            return pieces

        def ffn_chunk(c):
            for p in ffn_pieces(c):
                p()

        # ---------------- emission schedule ----------------
        def stats_group(g):
            for t in range(4 * g, 4 * g + 4):
                ln_stats(x_t[t], mvs1, t)
            ln_rstd(mvs1, rst1, 4 * g, 4)

        def kq(g, pr):
            return lambda: (k_group(g, (pr,)), q_slot(g, (pr,)))

        def _full_schedule():
            stats_group(0)
            for t in range(4):
                ln_tile(t)
            k_group(0, (0, 1))
            q_slot(0, (0, 1))
            att_head_pair(0, 0, [lambda: stats_group(1), lambda: ln_tile(4)])
            att_head_pair(0, 2, [lambda: ln_tile(5), kq(0, 2)])
            att_head_pair(0, 4, [lambda: ln_tile(6), lambda: ln_tile(7),
                                 kq(0, 3)])
            att_head_pair(0, 6, [kq(1, 0), kq(1, 1)])
            att_head_pair(1, 0, [lambda: stats_group(2), lambda: ln_tile(8),
                                 kq(1, 2)])
            att_head_pair(1, 2, [lambda: ln_tile(9), kq(1, 3)])
            att_head_pair(1, 4, [lambda: ln_tile(10), lambda: ln_tile(11),
                                 kq(2, 0)])
            att_head_pair(1, 6, [kq(2, 1), lambda: stats_group(3), kq(2, 2)])
            att_head_pair(2, 0, [lambda: ln_tile(12), kq(2, 3),
                                 lambda: ln_tile(13)])
            att_head_pair(2, 2, [lambda: ln_tile(14), lambda: ln_tile(15),
                                 lambda: epilogue_proj(0)])
            att_head_pair(2, 4, [lambda: epilogue_proj(1),
                                 lambda: ln_rstd(mvs2, rst2, 0, 2),
                                 lambda: epilogue_ln2(0), kq(3, 0)])
            att_head_pair(2, 6, [lambda: epilogue_ln2(1), kq(3, 1),
                                 kq(3, 2), lambda: epilogue_proj(2)])
            f0 = ffn_pieces(0)
            att_head_pair(3, 0, [kq(3, 3), lambda: epilogue_proj(3),
                                 lambda: ln_rstd(mvs2, rst2, 2, 2),
                                 lambda: epilogue_ln2(2),
                                 lambda: epilogue_ln2(3),
                                 f0[0], f0[1], f0[2]])
            f1p = ffn_pieces(1)
            att_head_pair(3, 2, [f0[3], f0[4], f0[5], f0[6],
                                 lambda: epilogue_proj(4),
                                 lambda: (epilogue_proj(5),
                                          ln_rstd(mvs2, rst2, 4, 2)),
                                 f1p[0], f1p[1]])
            f2 = ffn_pieces(2)
            att_head_pair(3, 4, [f1p[2], f1p[3], f1p[4], f1p[5], f1p[6],
                                 lambda: epilogue_ln2(4),
                                 lambda: epilogue_ln2(5), f2[0]])
            att_head_pair(3, 6, [f2[1], f2[2], f2[3], f2[4], f2[5], f2[6]])
            # tail (epilogues 6/7 read oq written by pair(3,6)'s tail)
            epilogue_proj(6)
            epilogue_proj(7)
            ln_rstd(mvs2, rst2, 6, 2)
            epilogue_ln2(6)
            epilogue_ln2(7)
            ffn_chunk(3)

        STAGE = int(_os.environ.get("STAGE", "8"))
        if STAGE == 12:
            # stage 8 + FFN chunks 0/1 at the proven interleave point
            for g in range(4):
                stats_group(g)
            for t in range(16):
                ln_tile(t)
            for g in range(4):
                k_group(g)
                q_slot(g)
            for s in range(3):
                for hp in range(4):
                    att_head_pair(s, 2 * hp)
            for t in range(4):
                epilogue_proj(t)
                if t % 2 == 1:
                    ln_rstd(mvs2, rst2, t - 1, 2)
            for t in range(4):
                epilogue_ln2(t)
            ffn_chunk(0)
            ffn_chunk(1)
            for hp in range(4):
                att_head_pair(3, 2 * hp)
            for t in range(4, OWN_NT):
                epilogue_proj(t)
                if t % 2 == 1:
                    ln_rstd(mvs2, rst2, t - 1, 2)
            for t in range(4, OWN_NT):
                epilogue_ln2(t)
            ffn_chunk(2)
            ffn_chunk(3)
        elif STAGE == 13:
            # stage 8 + only ffn chunk 0 at the interleave point
            for g in range(4):
                stats_group(g)
            for t in range(16):
                ln_tile(t)
            for g in range(4):
                k_group(g)
                q_slot(g)
            for s in range(3):
                for hp in range(4):
                    att_head_pair(s, 2 * hp)
            for t in range(4):
                epilogue_proj(t)
                if t % 2 == 1:
                    ln_rstd(mvs2, rst2, t - 1, 2)
            for t in range(4):
                epilogue_ln2(t)
            ffn_chunk(0)
            for hp in range(4):
                att_head_pair(3, 2 * hp)
            ffn_chunk(1)
            for t in range(4, OWN_NT):
                epilogue_proj(t)
                if t % 2 == 1:
                    ln_rstd(mvs2, rst2, t - 1, 2)
            for t in range(4, OWN_NT):
                epilogue_ln2(t)
            ffn_chunk(2)
            ffn_chunk(3)
        elif STAGE == 11:
            # between-slot interleave points only (no within-slot emissions)
            for g in range(4):
                stats_group(g)
            for t in range(16):
                ln_tile(t)
            for g in range(4):
                k_group(g)
                q_slot(g)
            for hp in range(4):
                att_head_pair(0, 2 * hp)
            for hp in range(4):
                att_head_pair(1, 2 * hp)
            epilogue_proj(0); epilogue_proj(1); ln_rstd(mvs2, rst2, 0, 2)
            epilogue_ln2(0); epilogue_ln2(1)
            for hp in range(4):
                att_head_pair(2, 2 * hp)
            epilogue_proj(2); epilogue_proj(3); ln_rstd(mvs2, rst2, 2, 2)
            epilogue_ln2(2); epilogue_ln2(3)
            ffn_chunk(0)
            for hp in range(4):
                att_head_pair(3, 2 * hp)
            epilogue_proj(4); epilogue_proj(5); ln_rstd(mvs2, rst2, 4, 2)
            epilogue_ln2(4); epilogue_ln2(5)
            ffn_chunk(1)
            epilogue_proj(6); epilogue_proj(7); ln_rstd(mvs2, rst2, 6, 2)
            epilogue_ln2(6); epilogue_ln2(7)
            ffn_chunk(2)
            ffn_chunk(3)
        elif STAGE == 10:
            # QKV upfront; epilogue/FFN interleaved after each slot and
            # between slot-3 pairs
            for g in range(4):
                stats_group(g)
            for t in range(16):
                ln_tile(t)
            for g in range(4):
                k_group(g)
                q_slot(g)
            for hp in range(4):
                att_head_pair(0, 2 * hp)
            for hp in range(4):
                att_head_pair(1, 2 * hp)
            epilogue_proj(0); epilogue_proj(1); ln_rstd(mvs2, rst2, 0, 2)
            epilogue_ln2(0); epilogue_ln2(1)
            att_head_pair(2, 0)
            att_head_pair(2, 2)
            epilogue_proj(2); epilogue_proj(3); ln_rstd(mvs2, rst2, 2, 2)
            att_head_pair(2, 4)
            epilogue_ln2(2); epilogue_ln2(3)
            att_head_pair(2, 6)
            ffn_chunk(0)
            att_head_pair(3, 0)
            epilogue_proj(4); epilogue_proj(5); ln_rstd(mvs2, rst2, 4, 2)
            att_head_pair(3, 2)
            epilogue_ln2(4); epilogue_ln2(5)
            att_head_pair(3, 4)
            ffn_chunk(1)
            att_head_pair(3, 6)
            ffn_chunk(2)
            epilogue_proj(6); epilogue_proj(7); ln_rstd(mvs2, rst2, 6, 2)
            epilogue_ln2(6); epilogue_ln2(7)
            ffn_chunk(3)
        elif STAGE == 8:
            # stage 4 plus ONE interleave point: slot 0-1 epilogues before slot3
            for g in range(4):
                stats_group(g)
            for t in range(16):
                ln_tile(t)
            for g in range(4):
                k_group(g)
                q_slot(g)
            for s in range(3):
                for hp in range(4):
                    att_head_pair(s, 2 * hp)
            for t in range(4):
                epilogue_proj(t)
                if t % 2 == 1:
                    ln_rstd(mvs2, rst2, t - 1, 2)
            for t in range(4):
                epilogue_ln2(t)
            for hp in range(4):
                att_head_pair(3, 2 * hp)
            for t in range(4, OWN_NT):
                epilogue_proj(t)
                if t % 2 == 1:
                    ln_rstd(mvs2, rst2, t - 1, 2)
            for t in range(4, OWN_NT):
                epilogue_ln2(t)
            for c in range(4):
                ffn_chunk(c)
        elif STAGE == 7:
            # QKV fully upfront; epilogue/FFN woven intra-pair into slots 2-3
            for g in range(4):
                stats_group(g)
            for t in range(16):
                ln_tile(t)
            for g in range(4):
                k_group(g)
                q_slot(g)
            for s in range(2):
                for hp in range(4):
                    att_head_pair(s, 2 * hp)
            att_head_pair(2, 0)
            att_head_pair(2, 2)
            att_head_pair(2, 4, [lambda: epilogue_proj(0),
                                 lambda: epilogue_proj(1),
                                 lambda: ln_rstd(mvs2, rst2, 0, 2)])
            att_head_pair(2, 6, [lambda: epilogue_ln2(0),
                                 lambda: epilogue_ln2(1),
                                 lambda: epilogue_proj(2)])
            f0 = ffn_pieces(0)
            att_head_pair(3, 0, [lambda: (epilogue_proj(3),
                                          ln_rstd(mvs2, rst2, 2, 2)),
                                 lambda: epilogue_ln2(2),
                                 lambda: epilogue_ln2(3),
                                 f0[0], f0[1], f0[2]])
            f1p = ffn_pieces(1)
            att_head_pair(3, 2, [f0[3], f0[4], f0[5], f0[6],
                                 lambda: epilogue_proj(4),
                                 lambda: (epilogue_proj(5),
                                          ln_rstd(mvs2, rst2, 4, 2)),
                                 f1p[0], f1p[1]])
            f2 = ffn_pieces(2)
            att_head_pair(3, 4, [f1p[2], f1p[3], f1p[4], f1p[5], f1p[6],
                                 lambda: epilogue_ln2(4),
                                 lambda: epilogue_ln2(5), f2[0]])
            att_head_pair(3, 6, [f2[1], f2[2], f2[3], f2[4], f2[5], f2[6]])
            epilogue_proj(6)
            epilogue_proj(7)
            ln_rstd(mvs2, rst2, 6, 2)
            epilogue_ln2(6)
            epilogue_ln2(7)
            ffn_chunk(3)
        elif STAGE == 6:
            # between-pair interleaving (no intra-pair weaving)
            stats_group(0)
            for t in range(4):
                ln_tile(t)
            k_group(0, (0, 1))
            q_slot(0, (0, 1))
            att_head_pair(0, 0)
            stats_group(1); ln_tile(4)
            att_head_pair(0, 2)
            ln_tile(5); kq(0, 2)()
            att_head_pair(0, 4)
            ln_tile(6); ln_tile(7); kq(0, 3)()
            att_head_pair(0, 6)
            kq(1, 0)(); kq(1, 1)()
            att_head_pair(1, 0)
            stats_group(2); ln_tile(8); kq(1, 2)()
            att_head_pair(1, 2)
            ln_tile(9); kq(1, 3)()
            att_head_pair(1, 4)
            ln_tile(10); ln_tile(11); kq(2, 0)()
            att_head_pair(1, 6)
            kq(2, 1)(); stats_group(3); kq(2, 2)()
            att_head_pair(2, 0)
            ln_tile(12); ln_tile(13); kq(2, 3)()
            att_head_pair(2, 2)
            ln_tile(14); ln_tile(15)
            att_head_pair(2, 4)
            epilogue_proj(0); epilogue_proj(1); ln_rstd(mvs2, rst2, 0, 2)
            att_head_pair(2, 6)
            epilogue_ln2(0); epilogue_ln2(1); kq(3, 0)(); kq(3, 1)()
            att_head_pair(3, 0)
            kq(3, 2)(); kq(3, 3)()
            epilogue_proj(2); epilogue_proj(3); ln_rstd(mvs2, rst2, 2, 2)
            att_head_pair(3, 2)
            epilogue_ln2(2); epilogue_ln2(3)
            ffn_chunk(0)
            att_head_pair(3, 4)
            ffn_chunk(1)
            epilogue_proj(4); epilogue_proj(5); ln_rstd(mvs2, rst2, 4, 2)
            att_head_pair(3, 6)
            epilogue_ln2(4); epilogue_ln2(5)
            ffn_chunk(2)
            epilogue_proj(6); epilogue_proj(7); ln_rstd(mvs2, rst2, 6, 2)
            epilogue_ln2(6); epilogue_ln2(7)
            ffn_chunk(3)
        elif STAGE == 5:
            # full ln/kq weave; epilogues+FFN sequential
            stats_group(0)
            for t in range(4):
                ln_tile(t)
            k_group(0, (0, 1))
            q_slot(0, (0, 1))
            att_head_pair(0, 0, [lambda: stats_group(1), lambda: ln_tile(4)])
            att_head_pair(0, 2, [lambda: ln_tile(5), kq(0, 2)])
            att_head_pair(0, 4, [lambda: ln_tile(6), lambda: ln_tile(7),
                                 kq(0, 3)])
            att_head_pair(0, 6, [kq(1, 0), kq(1, 1)])
            att_head_pair(1, 0, [lambda: stats_group(2), lambda: ln_tile(8),
                                 kq(1, 2)])
            att_head_pair(1, 2, [lambda: ln_tile(9), kq(1, 3)])
            att_head_pair(1, 4, [lambda: ln_tile(10), lambda: ln_tile(11),
                                 kq(2, 0)])
            att_head_pair(1, 6, [kq(2, 1), lambda: stats_group(3), kq(2, 2)])
            att_head_pair(2, 0, [lambda: ln_tile(12), kq(2, 3),
                                 lambda: ln_tile(13)])
            att_head_pair(2, 2, [lambda: ln_tile(14), lambda: ln_tile(15)])
            att_head_pair(2, 4, [kq(3, 0), kq(3, 1)])
            att_head_pair(2, 6, [kq(3, 2), kq(3, 3)])
            for hp in range(4):
                att_head_pair(3, 2 * hp)
            for t in range(OWN_NT):
                epilogue_proj(t)
                if t % 2 == 1:
                    ln_rstd(mvs2, rst2, t - 1, 2)
            for t in range(OWN_NT):
                epilogue_ln2(t)
            for c in range(4):
                ffn_chunk(c)
        elif STAGE <= 4:
            # reduced program for HW bisection
            for g in range(4):
                stats_group(g)
            for t in range(16):
                ln_tile(t)
            for g in range(4):
                k_group(g)
                q_slot(g)
            if STAGE >= 2:
                for s in range(4):
                    for hp in range(4):
                        att_head_pair(s, 2 * hp)
            if STAGE >= 3:
                for t in range(OWN_NT):
                    epilogue_proj(t)
                    if t % 2 == 1:
                        ln_rstd(mvs2, rst2, t - 1, 2)
                for t in range(OWN_NT):
                    epilogue_ln2(t)
            if STAGE >= 4:
                for c in range(4):
                    ffn_chunk(c)
            else:
                for t in range(OWN_NT):
                    o_t = opool.tile([128, D], FP32, tag="ot", name="ot")
                    nc.vector.tensor_copy(
                        o_t[:], (x2 if STAGE >= 3 else xb)[t][:])
                    nc.sync.dma_start(out_dram[t * 128:(t + 1) * 128, :],
                                      o_t[:])
        else:
            _full_schedule()

    import os
    if not os.environ.get("SKIP_WAITFIX"):
        _split_multi_waits(nc)
    return nc


_NC_CACHE = None


def _get_nc():
    global _NC_CACHE
    if _NC_CACHE is None:
        _NC_CACHE = _build_program()
    return _NC_CACHE


# ---------------------------------------------------------------------------
# Host side
# ---------------------------------------------------------------------------
def _to_fp8(a):
    return np.clip(a, -240.0, 240.0).astype(F8NP)


def _pack_dr(W):
    """[512, N] -> [2, 128, 2, N] fp8 with W scaled by WS."""
    Ws = np.asarray(W, np.float64) * WS
    out = np.empty((2, 128, 2, Ws.shape[1]), np.float64)
    for kp in range(2):
        for j in range(2):
            out[kp, :, j, :] = Ws[kp * 256 + j * 128:kp * 256 + (j + 1) * 128, :]
    return _to_fp8(out)


def _fold_weights(Wq, bq, Wk, bk, Wv, bv, Wp, bp, W1, b1, W2, b2, g1, be1,
                  g2, be2):
    f64 = np.float64
    Wq_e = (g1.astype(f64)[None, :, None] * Wq.astype(f64))      # [H,D,DK]
    Wk_e = (g1.astype(f64)[None, :, None] * Wk.astype(f64))
    Wv_e = (g1.astype(f64)[None, :, None] * Wv.astype(f64))
    bq_e = bq.astype(f64) + np.einsum("d,hdk->hk", be1.astype(f64), Wq.astype(f64))
    bk_e = bk.astype(f64) + np.einsum("d,hdk->hk", be1.astype(f64), Wk.astype(f64))
    bv_e = bv.astype(f64) + np.einsum("d,hdk->hk", be1.astype(f64), Wv.astype(f64))
    W1_e = g2.astype(f64)[:, None] * W1.astype(f64)
    b1_e = b1.astype(f64) + be2.astype(f64) @ W1.astype(f64)

    def head_major(W):  # [H,D,DK] -> [D, H*DK]
        return np.transpose(W, (1, 0, 2)).reshape(D, H * DK)

    out = {}
    out["wq8"] = _pack_dr(head_major(Wq_e))
    out["wk8"] = _pack_dr(head_major(Wk_e))
    out["wv8"] = _pack_dr(head_major(Wv_e))
    # wp8: quad g covers heads 4g..4g+3; [128(2 heads x 64k), 2(head pair), D]
    Wp_f = Wp.astype(f64).reshape(H, DK, D)
    wp8 = np.empty((2, 128, 2, D), np.float64)
    for g in range(2):
        for j in range(2):
            h0 = 4 * g + 2 * j
            wp8[g, 0:64, j, :] = Wp_f[h0] * WS
            wp8[g, 64:128, j, :] = Wp_f[h0 + 1] * WS
    out["wp8"] = _to_fp8(wp8)
    out["w18"] = _pack_dr(W1_e)
    # w28: [8 hidpair, 128, 2, D]
    W2_f = W2.astype(f64) * WS
    w28 = np.empty((8, 128, 2, D), np.float64)
    for hp in range(8):
        for j in range(2):
            w28[hp, :, j, :] = W2_f[hp * 256 + j * 128:hp * 256 + (j + 1) * 128, :]
    out["w28"] = _to_fp8(w28)

    bqk = np.zeros((128, 8), np.float32)
    for pr in range(4):
        bqk[:, pr] = np.concatenate([bq_e[2 * pr], bq_e[2 * pr + 1]])
        bqk[:, 4 + pr] = np.concatenate([bk_e[2 * pr], bk_e[2 * pr + 1]])
    out["bqk"] = bqk
    out["b1s"] = np.ascontiguousarray(
        (WS * b1_e).reshape(16, 128).T).astype(np.float32)
    out["bv_row"] = bv_e.reshape(1, H * DK).astype(np.float32)
    out["bp_row"] = bp.reshape(1, D).astype(np.float32)
    out["b2_row"] = b2.reshape(1, D).astype(np.float32)
    return out


def _build_masks(p):
    """[4, 128, 2, 1024] fp8: per slot, DR rhs for the last quad's mask."""
    perm = PERM[p]
    masks = np.zeros((4, 128, 2, 1024), np.float32)
    for s in range(4):
        U = SPANS[s]
        qc = perm[OWN_POS[s]]              # original own chunk for this slot
        q_tok = qc * 256 + np.arange(256)  # original q token ids
        for j in range(4):
            u = (U - 4) + j                # x' kv tile index
            kv_tok = perm[u // 2] * 256 + (u % 2) * 128 + np.arange(128)
            m = np.where(kv_tok[:, None] <= q_tok[None, :], 0.0, NEG)
            masks[s, :, 0, j * 256:(j + 1) * 256] = m
    return masks.astype(F8NP)


def kernel(x, Wq, bq, Wk, bk, Wv, bv, Wp, bp, W1, b1, W2, b2, g1, be1, g2, be2):
    x = np.asarray(x, np.float32)
    folded = _fold_weights(
        np.asarray(Wq), np.asarray(bq), np.asarray(Wk), np.asarray(bk),
        np.asarray(Wv), np.asarray(bv), np.asarray(Wp), np.asarray(bp),
        np.asarray(W1), np.asarray(b1), np.asarray(W2), np.asarray(b2),
        np.asarray(g1), np.asarray(be1), np.asarray(g2), np.asarray(be2))

    masks_by_p = [_build_masks(0), _build_masks(1)]
    in_maps = []
    for c in range(8):
        b, p = c // 2, c % 2
        perm = PERM[p]
        xp = np.concatenate([x[b, pc * 256:(pc + 1) * 256] for pc in perm])
        m = dict(folded)
        m["xp"] = np.ascontiguousarray(xp)
        m["masks8"] = masks_by_p[p]
        in_maps.append(m)

    nc = _get_nc()
    res = run_bass_kernel_spmd(nc, in_maps, list(range(8)))

    out = np.empty((B, T, D), np.float32)
    for c in range(8):
        b, p = c // 2, c % 2
        perm = PERM[p]
        o = res.results[c]["out"]
        for s in range(4):
            oc = perm[OWN_POS[s]]
            out[b, oc * 256:(oc + 1) * 256] = o[s * 256:(s + 1) * 256]
    return out


# revision 7
# speedup vs baseline: 1.0175x; 1.0114x over previous
"""Trainium2 Bass kernel for a dense transformer block (nn_Block_83880711291003).

Full (unsharded) inputs in, full output out. 8 NeuronCores:
  core c -> batch b = c//2, parity p = c%2.
Each core computes LN1 + K/V over its batch's full 2048 tokens and owns 1024
query tokens (4 chunks of 256). A host-side chunk permutation places each
parity's own chunks at fixed x'-positions {1,3,5,7} so ONE SPMD program works
for both parities: attention slot s (span 4/8/12/16 kv-tiles over the x'
prefix) handles the own chunk at x' position 2s+1; causality is enforced by
host-built additive masks folded into the score PSUM via DoubleRow identity
matmuls. Heavy matmuls (QKV, AV, out-proj, FFN) run as fp8e4m3 DoubleRow
(256-deep contraction, 0.5 cycles/col); scores stay bf16.
"""

import sys

for _p in ("/opt/trn_rl_repo", "/root/.axon_site/_ro/trn_rl_repo"):
    if _p not in sys.path:
        sys.path.append(_p)

from contextlib import ExitStack

import ml_dtypes
import numpy as np

import concourse.bass as bass
import concourse.tile as tile
from concourse import mybir
from concourse.bass_utils import run_bass_kernel_spmd
from concourse.masks import make_identity
from concourse.vector_clock import ScopedClock

FP32 = mybir.dt.float32
BF16 = mybir.dt.bfloat16
FP8 = mybir.dt.float8e4
BFNP = ml_dtypes.bfloat16
F8NP = ml_dtypes.float8_e4m3

ACT = mybir.ActivationFunctionType
ALU = mybir.AluOpType
DR = mybir.MatmulPerfMode.DoubleRow

B, T, D = 4, 2048, 512
H, DK = 8, 64
FF = 4 * D
EPS = 1e-5
WS = 16.0          # fp8 weight scale
IWS = 1.0 / WS
NEG = -240.0       # additive mask value (fp8e4m3 max normal magnitude)
SPANS = (4, 8, 12, 16)      # kv span per slot, in 128-tiles
OWN_POS = (1, 3, 5, 7)      # x' chunk position of each slot's own q chunk
# per-parity x' chunk order: x' position i holds original chunk PERM[p][i]
PERM = ((1, 0, 2, 3, 5, 4, 6, 7), (0, 1, 3, 2, 4, 5, 7, 6))
OWN_T = 1024
OWN_NT = 8

# ---------------------------------------------------------------------------
# Workaround: this walrus build rejects >1 semaphore wait per instruction.
# ---------------------------------------------------------------------------
_uid = [0]


def _split_multi_waits(nc):
    for blk in nc.m.functions[0].blocks:
        insts = list(blk.instructions)
        out, changed = [], False
        for inst in insts:
            si = inst.sync_info
            waits = list(si.on_wait) if si else []
            if len(waits) > 1:
                changed = True
                for w in waits[:-1]:
                    _uid[0] += 1
                    nop = mybir.InstNoOp(name=f"I-waitfix-{_uid[0]}", ins=[], outs=[])
                    nop.engine = inst.engine
                    nop.sync_info = mybir.SyncInfo(on_wait=[w], on_update=[])
                    out.append(nop)
                inst.sync_info = mybir.SyncInfo(
                    on_wait=[waits[-1]], on_update=list(si.on_update)
                )
            out.append(inst)
        if changed:
            blk.instructions = out


def _patched_drain_and_barrier(self, tick_clock, wait_clock):
    nc = self.nc
    probe = nc.sync.nop()
    wait_clock.add_sem_waits(probe.ins, ScopedClock({None: tick_clock.global_clock}))
    nc.sync.drain()
    nc.all_engine_barrier()
    popped = nc._tile_sem_poison_stack.pop()
    assert popped is self._sem_poison
    nc.clear_and_free_semaphores(list(self.sems.allocated().values()))
    nc.all_engine_barrier()


tile.TileContext._drain_and_barrier = _patched_drain_and_barrier


# ---------------------------------------------------------------------------
# Device program (identical on all 8 cores)
# ---------------------------------------------------------------------------
import os as _os
CFG = {
    "ht_early": _os.environ.get("CFG_HT_EARLY", "act"),   # hT8 copies t<4
    "ht_mid": _os.environ.get("CFG_HT_MID", "act"),       # hT8 copies t>=4
    "h2t": _os.environ.get("CFG_H2T", "dve"),             # h2T8 copies
    "oq_late_act": _os.environ.get("CFG_OQ", "0") == "1", # oq s>=2 on ACT
    "kq0_act": _os.environ.get("CFG_KQ0", "1") == "1",    # kq g0/s0 on ACT
}


def _build_program():
    nc = bass.Bass("TRN2", target_bir_lowering=False, debug=False)

    din = {}
    for name, shape, dt in [
        ("xp", [T, D], FP32),                 # permuted x for this core
        ("wq8", [2, 128, 2, D], FP8),
        ("wk8", [2, 128, 2, D], FP8),
        ("wv8", [2, 128, 2, D], FP8),
        ("wp8", [2, 128, 2, D], FP8),         # [quad A/B, p, j, d]
        ("w18", [2, 128, 2, FF], FP8),
        ("w28", [8, 128, 2, D], FP8),
        ("bqk", [128, 8], FP32),              # cols 0-3 bq per pr; 4-7 bk
        ("b1s", [128, 16], FP32),             # WS * b1 per hidden tile
        ("bv_row", [1, D], FP32),
        ("bp_row", [1, D], FP32),
        ("b2_row", [1, D], FP32),
        ("masks8", [4, 128, 2, 1024], FP8),   # per slot: DR mask rhs (j1=0)
    ]:
        din[name] = nc.dram_tensor(name, shape, dt, kind="ExternalInput").ap()
    out_dram = nc.dram_tensor("out", [OWN_T, D], FP32, kind="ExternalOutput").ap()

    with tile.TileContext(nc) as tc, ExitStack() as ctx:
        P = ctx.enter_context

        wpool = P(tc.tile_pool(name="weights", bufs=1))
        persist = P(tc.tile_pool(name="persist", bufs=1))
        xio = P(tc.tile_pool(name="xio",
                              bufs=int(_os.environ.get("CFG_XIO", "8"))))
        small = P(tc.tile_pool(name="small", bufs=int(_os.environ.get("CFG_SM", "6"))))
        hpool = P(tc.tile_pool(name="htok",
                               bufs=int(_os.environ.get("CFG_HP", "4"))))
        ppool = P(tc.tile_pool(name="pT",
                               bufs=int(_os.environ.get("CFG_PT", "6"))))
        opool = P(tc.tile_pool(name="outio", bufs=int(_os.environ.get("CFG_OP", "3"))))
        f18pool = P(tc.tile_pool(name="f18", bufs=int(_os.environ.get("CFG_F18", "2"))))
        psS = P(tc.tile_pool(name="psS", bufs=2, space="PSUM"))   # [128,1024]
        psA = P(tc.tile_pool(name="psA", bufs=2, space="PSUM"))   # AV accum
        psW = P(tc.tile_pool(name="psW", bufs=2, space="PSUM"))   # [128,512]

        # ---- input DMAs: x tiles interleaved with weights (SP queue);
        # masks/broadcasts/FFN weights via gpsimd (SWDGE) ----
        x_t = [None] * 16
        own_slot = {2: 0, 3: 1, 6: 2, 7: 3, 10: 4, 11: 5, 14: 6, 15: 7}
        wq8 = [wpool.tile([128, 2, D], FP8, tag=f"wq8{k}", name=f"wq8{k}")
               for k in range(2)]
        wk8 = [wpool.tile([128, 2, D], FP8, tag=f"wk8{k}", name=f"wk8{k}")
               for k in range(2)]
        wv8 = [wpool.tile([128, 2, D], FP8, tag=f"wv8{k}", name=f"wv8{k}")
               for k in range(2)]
        wp8 = [wpool.tile([128, 2, D], FP8, tag=f"wp8{k}", name=f"wp8{k}")
               for k in range(2)]

        def dma_x(t):
            if t in own_slot:
                x_t[t] = persist.tile([128, D], FP32, tag=f"xo{t}", name=f"xo{t}")
            else:
                x_t[t] = xio.tile([128, D], FP32, tag="xin", name="xin")
            nc.sync.dma_start(x_t[t][:], din["xp"][t * 128:(t + 1) * 128, :])

        bqk = wpool.tile([128, 8], FP32, tag="bqk", name="bqk")
        b1s = wpool.tile([128, 16], FP32, tag="b1s", name="b1s")
        for t in range(4):
            dma_x(t)
        for k in range(2):
            nc.sync.dma_start(wv8[k][:], din["wv8"][k])
            nc.sync.dma_start(wk8[k][:], din["wk8"][k])
        for t in range(4, 8):
            dma_x(t)
        for k in range(2):
            nc.sync.dma_start(wq8[k][:], din["wq8"][k])
        nc.sync.dma_start(bqk[:], din["bqk"][:])
        for t in range(8, 16):
            dma_x(t)
        for k in range(2):
            nc.sync.dma_start(wp8[k][:], din["wp8"][k])
        nc.sync.dma_start(b1s[:], din["b1s"][:])

        # constants first: they gate the first transposes/LN ops and must not
        # queue behind SWDGE descriptor generation on the Pool engine
        ident8 = wpool.tile([128, 128], FP8, tag="ident8", name="ident8")
        make_identity(nc, ident8[:])
        identB = wpool.tile([128, 128], BF16, tag="identB", name="identB")
        make_identity(nc, identB[:])
        identDR = wpool.tile([128, 2, 128], FP8, tag="identDR", name="identDR")
        nc.vector.memset(identDR[:, 1, :], 0.0)
        make_identity(nc, identDR[:, 0, :])
        eps_t = wpool.tile([128, 1], FP32, tag="eps", name="eps")
        nc.vector.memset(eps_t[:], EPS)
        # preload ACT tables for Exp/Relu/Sqrt before the hot loops
        warm = wpool.tile([128, 1], FP32, tag="warm", name="warm")
        nc.scalar.activation(out=warm[:], in_=eps_t[:], func=ACT.Exp, scale=1.0)
        nc.scalar.activation(out=warm[:], in_=eps_t[:], func=ACT.Relu, scale=1.0)
        nc.scalar.activation(out=warm[:], in_=eps_t[:], func=ACT.Sqrt, scale=1.0)

        def bcast_row(name):
            t = wpool.tile([128, D], FP32, tag=f"bc_{name}", name=f"bc_{name}")
            src = din[name]
            ap = bass.AP(tensor=src.tensor, offset=src.offset,
                         ap=[[0, 128], src.ap[1]])
            nc.scalar.dma_start(out=t[:], in_=ap)
            return t

        bv_b = bcast_row("bv_row")
        bp_b = bcast_row("bp_row")
        b2_b = bcast_row("b2_row")

        masks8 = wpool.tile([128, 4, 2, 1024], FP8, tag="masks8", name="masks8")
        nc.sync.dma_start(masks8[:], din["masks8"].rearrange("s p j f -> p s j f"))
        w18 = [wpool.tile([128, 2, FF], FP8, tag=f"w18{k}", name=f"w18{k}")
               for k in range(2)]
        w28 = [wpool.tile([128, 2, D], FP8, tag=f"w28{k}", name=f"w28{k}")
               for k in range(8)]
        for k in range(2):
            nc.sync.dma_start(w18[k][:], din["w18"][k])
        for k in range(8):
            nc.sync.dma_start(w28[k][:], din["w28"][k])

        # ---- long-lived activations ----
        hT8 = [persist.tile([128, 2, T], FP8, tag=f"hT8{k}", name=f"hT8{k}")
               for k in range(2)]
        kT = [persist.tile([128, T], BF16, tag=f"kT{pr}", name=f"kT{pr}")
              for pr in range(4)]
        qT = [persist.tile([128, 4, 256], BF16, tag=f"qT{pr}", name=f"qT{pr}")
              for pr in range(4)]
        # v8[p, kvpair, j, h, k]; col 64 is the ones column (denominator)
        v8 = persist.tile([128, 8, 2, H, 65], FP8, tag="v8", name="v8")
        nc.vector.memset(v8[:, :, :, :, 64], 1.0)
        oq = [persist.tile([128, 2, OWN_T], FP8, tag=f"oq{g}", name=f"oq{g}")
              for g in range(2)]
        xb = [persist.tile([128, D], FP32, tag=f"xb{t}", name=f"xb{t}")
              for t in range(OWN_NT)]
        x2 = [persist.tile([128, D], FP32, tag=f"x2_{t}", name=f"x2_{t}")
              for t in range(OWN_NT)]
        h2T8 = [persist.tile([128, 2, OWN_T], FP8, tag=f"h2T8{k}", name=f"h2T8{k}")
                for k in range(2)]

        # LN statistics, batched: per-tile bn stats land in a shared [128,2,N]
        # tile; one Sqrt + one reciprocal per tile-group keeps ACT's queue
        # clear of per-tile LN work.
        mvs1 = persist.tile([128, 2, 16], FP32, tag="mvs1", name="mvs1")
        rst1 = persist.tile([128, 16], FP32, tag="rst1", name="rst1")
        mvs2 = persist.tile([128, 2, 8], FP32, tag="mvs2", name="mvs2")
        rst2 = persist.tile([128, 8], FP32, tag="rst2", name="rst2")

        def ln_stats(x_in, mvs, t):
            stats = small.tile([128, 6], FP32, tag="bnst", name="bnst")
            nc.vector.bn_stats(out=stats[:], in_=x_in[:])
            mv = small.tile([128, 2], FP32, tag="bnmv", name="bnmv")
            nc.vector.bn_aggr(out=mv[:], in_=stats[:])
            nc.vector.tensor_copy(mvs[:, :, t], mv[:])

        def ln_rstd(mvs, rst, t0, n):
            rs = small.tile([128, 4], FP32, tag="rs", name="rs")
            nc.scalar.activation(out=rs[:, 0:n], in_=mvs[:, 1, t0:t0 + n],
                                 func=ACT.Sqrt, bias=eps_t[:], scale=1.0)
            nc.vector.reciprocal(out=rst[:, t0:t0 + n], in_=rs[:, 0:n])

        def ln_apply(x_in, h_out, mvs, rst, t, eng=None):
            eng = eng or nc.vector
            eng.tensor_scalar(
                out=h_out[:], in0=x_in[:], scalar1=mvs[:, 0, t:t + 1],
                scalar2=rst[:, t:t + 1], op0=ALU.subtract, op1=ALU.mult)

        def transpose_pairs(h_t, dst, dst_col, n, copy_eng="mix"):
            """dst[k][:, j, dst_col:+n] = h_t[:, (2k+j)*128:+128].T
            (bf16 transpose through PSUM, cast to fp8 on the copy out)."""
            for k in range(2):
                ps = psW.tile([128, 2, 128], BF16, tag="w", name="w")
                for j in range(2):
                    nc.tensor.matmul(ps[:, j, :],
                                     h_t[:, (2 * k + j) * 128:(2 * k + j + 1) * 128],
                                     identB[:], is_transpose=True,
                                     start=(j == 0), stop=(j == 1),
                                     skip_group_check=True)
                if copy_eng == "act" or (copy_eng == "mix" and k == 1):
                    nc.scalar.copy(dst[k][:, :, dst_col:dst_col + n], ps[:])
                else:
                    nc.vector.tensor_copy(dst[k][:, :, dst_col:dst_col + n],
                                          ps[:])

        # ---------------- stage A: LN1 + QKV per tile/group ----------------
        def ln_tile(t):
            h_t = hpool.tile([128, D], BF16, tag="h1", name="h1")
            ln_apply(x_t[t], h_t, mvs1, rst1, t,
                     nc.gpsimd if t >= 4 else nc.vector)
            transpose_pairs(h_t, hT8, t * 128, 128,
                            CFG["ht_early"] if t < 4 else CFG["ht_mid"])
            if t in own_slot:
                nc.gpsimd.tensor_add(xb[own_slot[t]][:], x_t[t][:], bp_b[:])
            # V projection for this tile
            ps = psW.tile([128, D], FP32, tag="w", name="w")
            for k in range(2):
                nc.tensor.matmul(ps[:], hT8[k][:, :, t * 128:(t + 1) * 128],
                                 wv8[k][:], start=(k == 0), stop=(k == 1),
                                 perf_mode=DR)
            nc.vector.scalar_tensor_tensor(
                out=v8[:, t // 2, t % 2, :, 0:64],
                in0=ps[:].rearrange("p (h k) -> p h k", h=H),
                scalar=IWS,
                in1=bv_b[:].rearrange("p (h k) -> p h k", h=H),
                op0=ALU.mult, op1=ALU.add)

        def k_group(g, prs=range(4)):
            """kT for 512-token group g (x' tiles 4g..4g+3)."""
            for pr in prs:
                ps = psW.tile([128, D], FP32, tag="w", name="w")
                for k in range(2):
                    nc.tensor.matmul(
                        ps[:], wk8[k][:, :, pr * 128:(pr + 1) * 128],
                        hT8[k][:, :, g * 512:(g + 1) * 512],
                        start=(k == 0), stop=(k == 1), perf_mode=DR)
                if g == 0 and CFG["kq0_act"]:
                    nc.scalar.activation(
                        out=kT[pr][:, g * 512:(g + 1) * 512], in_=ps[:],
                        func=ACT.Identity, bias=bqk[:, 4 + pr:5 + pr],
                        scale=IWS)
                else:
                    nc.vector.tensor_scalar(
                        out=kT[pr][:, g * 512:(g + 1) * 512], in0=ps[:],
                        scalar1=IWS, scalar2=bqk[:, 4 + pr:5 + pr],
                        op0=ALU.mult, op1=ALU.add)

        def q_slot(s, prs=range(4)):
            pos = OWN_POS[s]
            for pr in prs:
                ps = psW.tile([128, 256], FP32, tag="w", name="w")
                for k in range(2):
                    nc.tensor.matmul(
                        ps[:], wq8[k][:, :, pr * 128:(pr + 1) * 128],
                        hT8[k][:, :, pos * 256:(pos + 1) * 256],
                        start=(k == 0), stop=(k == 1), perf_mode=DR)
                if s == 0 and CFG["kq0_act"]:
                    nc.scalar.activation(
                        out=qT[pr][:, s, :], in_=ps[:], func=ACT.Identity,
                        bias=bqk[:, pr:pr + 1], scale=IWS)
                else:
                    nc.vector.tensor_scalar(
                        out=qT[pr][:, s, :], in0=ps[:],
                        scalar1=IWS, scalar2=bqk[:, pr:pr + 1],
                        op0=ALU.mult, op1=ALU.add)

        # ---------------- attention (software-pipelined head pair) --------
        def att_scores(s, h, q):
            """scores + mask + exp for quad q of (slot s, head h) -> pt8."""
            nquads = SPANS[s] // 4
            pr, sub = h // 2, h % 2
            krows = kT[pr][sub * 64:(sub + 1) * 64, :]
            qrows = qT[pr][sub * 64:(sub + 1) * 64, s, :]
            sps = psS.tile([128, 4, 256], FP32, tag="sps", name="sps")
            masked = (q == nquads - 1)
            for j in range(4):
                u = q * 4 + j
                nc.tensor.matmul(sps[:, j, :],
                                 krows[:, u * 128:(u + 1) * 128], qrows,
                                 start=(j % 2 == 0),
                                 stop=(j % 2 == 1 and not masked),
                                 skip_group_check=True)
            if masked:
                for half in range(2):
                    nc.tensor.matmul(
                        sps[:, 2 * half:2 * half + 2, :].rearrange(
                            "p j f -> p (j f)"),
                        identDR[:],
                        masks8[:, s, :, half * 512:(half + 1) * 512],
                        start=False, stop=True, perf_mode=DR,
                        skip_group_check=True)
            pt8 = ppool.tile([128, 4, 256], FP8, tag="pt8", name="pt8")
            nc.scalar.activation(out=pt8[:], in_=sps[:], func=ACT.Exp,
                                 scale=0.125)
            return pt8

        def att_av(s, h, q, pt8, ops, oi):
            nquads = SPANS[s] // 4
            for jp in range(2):
                kvp = q * 2 + jp
                for sb in range(2):
                    nc.tensor.matmul(
                        ops[:, 2 * oi + sb, :],
                        pt8[:, 2 * jp:2 * jp + 2, sb * 128:(sb + 1) * 128],
                        v8[:, kvp, :, h, :],
                        start=(q == 0 and jp == 0 and oi == 0 and sb == 0),
                        stop=(q == nquads - 1 and jp == 1),
                        perf_mode=DR, skip_group_check=True)

        def att_head_pair(s, h0, weave=()):
            """heads h0, h0+1 with scores/AV software pipelining; `weave` is a
            list of small zero-arg emitters drained at stage boundaries."""
            nquads = SPANS[s] // 4
            weave = list(weave)
            ops = psA.tile([128, 4, 65], FP32, tag="ops", name="ops")
            pend = []
            for q in range(nquads):
                pt_a = att_scores(s, h0, q)
                pend.append((h0, q, pt_a, 0))
                if len(pend) > 1:
                    hh, qq, pt, oi = pend.pop(0)
                    att_av(s, hh, qq, pt, ops, oi)
                if weave:
                    weave.pop(0)()
                pt_b = att_scores(s, h0 + 1, q)
                pend.append((h0 + 1, q, pt_b, 1))
                hh, qq, pt, oi = pend.pop(0)
                att_av(s, hh, qq, pt, ops, oi)
                if weave:
                    weave.pop(0)()
            hh, qq, pt, oi = pend.pop(0)
            att_av(s, hh, qq, pt, ops, oi)
            for fn in weave:
                fn()
            rc = small.tile([128, 4], FP32, tag="rc", name="rc")
            nc.vector.reciprocal(out=rc[:], in_=ops[:, :, 64])
            for sb in range(2):
                tcol = (s * 2 + sb) * 128
                ps = psW.tile([128, 128], BF16, tag="w", name="w")
                for i in range(2):
                    oS = small.tile([128, 64], BF16, tag="oS", name="oS")
                    nc.vector.tensor_scalar(
                        out=oS[:], in0=ops[:, 2 * i + sb, 0:64],
                        scalar1=rc[:, 2 * i + sb:2 * i + sb + 1],
                        scalar2=None, op0=ALU.mult)
                    nc.tensor.transpose(ps[i * 64:(i + 1) * 64, :], oS[:],
                                        identB[:])
                if (s >= 2 and CFG["oq_late_act"]) or sb == 1:
                    nc.scalar.copy(
                        oq[h0 // 4][:, (h0 % 4) // 2, tcol:tcol + 128], ps[:])
                else:
                    nc.vector.tensor_copy(
                        oq[h0 // 4][:, (h0 % 4) // 2, tcol:tcol + 128], ps[:])

        # ---------------- per-own-tile epilogue: proj + LN2 ----------------
        def epilogue_proj(t):
            ps = psW.tile([128, D], FP32, tag="w", name="w")
            for g in range(2):
                nc.tensor.matmul(ps[:], oq[g][:, :, t * 128:(t + 1) * 128],
                                 wp8[g][:], start=(g == 0), stop=(g == 1),
                                 perf_mode=DR)
            nc.vector.scalar_tensor_tensor(
                out=x2[t][:], in0=ps[:], scalar=IWS, in1=xb[t][:],
                op0=ALU.mult, op1=ALU.add)
            ln_stats(x2[t], mvs2, t)

        def epilogue_ln2(t):
            h2 = hpool.tile([128, D], BF16, tag="h2", name="h2")
            ln_apply(x2[t], h2, mvs2, rst2, t, nc.gpsimd)
            transpose_pairs(h2, h2T8, t * 128, 128, CFG["h2t"])

        # ---------------- FFN per 256-token own chunk ----------------
        def ffn2_tile(f8c, tt, t):
            ps = psS.tile([128, 512], FP32, tag="sps", name="sps")
            for k in range(8):
                nc.tensor.matmul(
                    ps[:],
                    f8c[:, 2 * k:2 * k + 2, tt * 128:(tt + 1) * 128],
                    w28[k][:], start=(k == 0), stop=(k == 7), perf_mode=DR)
            o_t = opool.tile([128, D], FP32, tag="ot", name="ot")
            nc.vector.scalar_tensor_tensor(
                out=o_t[:], in0=ps[:], scalar=1.0 / (WS * WS), in1=x2[t][:],
                op0=ALU.mult, op1=ALU.add)
            nc.gpsimd.tensor_add(o_t[:], o_t[:], b2_b[:])
            nc.sync.dma_start(out_dram[t * 128:(t + 1) * 128, :], o_t[:])

        def ffn1_quarter(c, f8c, qi):
            for ht in range(4 * qi, 4 * qi + 4):
                ps = psW.tile([128, 256], FP32, tag="w", name="w")
                for k in range(2):
                    nc.tensor.matmul(
                        ps[:], w18[k][:, :, ht * 128:(ht + 1) * 128],
                        h2T8[k][:, :, c * 256:(c + 1) * 256],
                        start=(k == 0), stop=(k == 1), perf_mode=DR)
                if c >= int(_os.environ.get("CFG_RELU_ACT_FROM", "1")):
                    nc.scalar.activation(out=f8c[:, ht, :], in_=ps[:],
                                         func=ACT.Relu, bias=b1s[:, ht:ht + 1],
                                         scale=1.0)
                else:
                    nc.vector.tensor_scalar(
                        out=f8c[:, ht, :], in0=ps[:],
                        scalar1=b1s[:, ht:ht + 1], scalar2=0.0,
                        op0=ALU.add, op1=ALU.max)

        def ffn_pieces(c):
            """ffn chunk c as a list of small weave-able emitters."""
            box = {}

            def mk_f8c():
                box["f"] = f18pool.tile([128, 16, 256], FP8, tag="f8", name="f8")

            pieces = [mk_f8c]
            for qi in range(4):
                pieces.append(lambda qi=qi: ffn1_quarter(c, box["f"], qi))
            for tt in range(2):
                pieces.append(lambda tt=tt: ffn2_tile(box["f"], tt, 2 * c + tt))
            return pieces

        def ffn_chunk(c):
            for p in ffn_pieces(c):
                p()

        # ---------------- emission schedule ----------------
        def stats_group(g):
            for t in range(4 * g, 4 * g + 4):
                ln_stats(x_t[t], mvs1, t)
            ln_rstd(mvs1, rst1, 4 * g, 4)

        def kq(g, pr):
            return lambda: (k_group(g, (pr,)), q_slot(g, (pr,)))

        def _full_schedule():
            stats_group(0)
            for t in range(4):
                ln_tile(t)
            k_group(0, (0, 1))
            q_slot(0, (0, 1))
            att_head_pair(0, 0, [lambda: stats_group(1), lambda: ln_tile(4)])
            att_head_pair(0, 2, [lambda: ln_tile(5), kq(0, 2)])
            att_head_pair(0, 4, [lambda: ln_tile(6), lambda: ln_tile(7),
                                 kq(0, 3)])
            att_head_pair(0, 6, [kq(1, 0), kq(1, 1)])
            att_head_pair(1, 0, [lambda: stats_group(2), lambda: ln_tile(8),
                                 kq(1, 2)])
            att_head_pair(1, 2, [lambda: ln_tile(9), kq(1, 3)])
            att_head_pair(1, 4, [lambda: ln_tile(10), lambda: ln_tile(11),
                                 kq(2, 0)])
            att_head_pair(1, 6, [kq(2, 1), lambda: stats_group(3), kq(2, 2)])
            att_head_pair(2, 0, [lambda: ln_tile(12), kq(2, 3),
                                 lambda: ln_tile(13)])
            att_head_pair(2, 2, [lambda: ln_tile(14), lambda: ln_tile(15),
                                 lambda: epilogue_proj(0)])
            att_head_pair(2, 4, [lambda: epilogue_proj(1),
                                 lambda: ln_rstd(mvs2, rst2, 0, 2),
                                 lambda: epilogue_ln2(0), kq(3, 0)])
            att_head_pair(2, 6, [lambda: epilogue_ln2(1), kq(3, 1),
                                 kq(3, 2), lambda: epilogue_proj(2)])
            f0 = ffn_pieces(0)
            att_head_pair(3, 0, [kq(3, 3), lambda: epilogue_proj(3),
                                 lambda: ln_rstd(mvs2, rst2, 2, 2),
                                 lambda: epilogue_ln2(2),
                                 lambda: epilogue_ln2(3),
                                 f0[0], f0[1], f0[2]])
            f1p = ffn_pieces(1)
            att_head_pair(3, 2, [f0[3], f0[4], f0[5], f0[6],
                                 lambda: epilogue_proj(4),
                                 lambda: (epilogue_proj(5),
                                          ln_rstd(mvs2, rst2, 4, 2)),
                                 f1p[0], f1p[1]])
            f2 = ffn_pieces(2)
            att_head_pair(3, 4, [f1p[2], f1p[3], f1p[4], f1p[5], f1p[6],
                                 lambda: epilogue_ln2(4),
                                 lambda: epilogue_ln2(5), f2[0]])
            att_head_pair(3, 6, [f2[1], f2[2], f2[3], f2[4], f2[5], f2[6]])
            # tail (epilogues 6/7 read oq written by pair(3,6)'s tail)
            epilogue_proj(6)
            epilogue_proj(7)
            ln_rstd(mvs2, rst2, 6, 2)
            epilogue_ln2(6)
            epilogue_ln2(7)
            ffn_chunk(3)

        STAGE = int(_os.environ.get("STAGE", "8"))
        if STAGE == 12:
            # stage 8 + FFN chunks 0/1 at the proven interleave point
            for g in range(4):
                stats_group(g)
            for t in range(16):
                ln_tile(t)
            for g in range(4):
                k_group(g)
                q_slot(g)
            for s in range(3):
                for hp in range(4):
                    att_head_pair(s, 2 * hp)
            for t in range(4):
                epilogue_proj(t)
                if t % 2 == 1:
                    ln_rstd(mvs2, rst2, t - 1, 2)
            for t in range(4):
                epilogue_ln2(t)
            ffn_chunk(0)
            ffn_chunk(1)
            for hp in range(4):
                att_head_pair(3, 2 * hp)
            for t in range(4, OWN_NT):
                epilogue_proj(t)
                if t % 2 == 1:
                    ln_rstd(mvs2, rst2, t - 1, 2)
            for t in range(4, OWN_NT):
                epilogue_ln2(t)
            ffn_chunk(2)
            ffn_chunk(3)
        elif STAGE == 13:
            # stage 8 + only ffn chunk 0 at the interleave point
            for g in range(4):
                stats_group(g)
            for t in range(16):
                ln_tile(t)
            for g in range(4):
                k_group(g)
                q_slot(g)
            for s in range(3):
                for hp in range(4):
                    att_head_pair(s, 2 * hp)
            for t in range(4):
                epilogue_proj(t)
                if t % 2 == 1:
                    ln_rstd(mvs2, rst2, t - 1, 2)
            for t in range(4):
                epilogue_ln2(t)
            ffn_chunk(0)
            for hp in range(4):
                att_head_pair(3, 2 * hp)
            ffn_chunk(1)
            for t in range(4, OWN_NT):
                epilogue_proj(t)
                if t % 2 == 1:
                    ln_rstd(mvs2, rst2, t - 1, 2)
            for t in range(4, OWN_NT):
                epilogue_ln2(t)
            ffn_chunk(2)
            ffn_chunk(3)
        elif STAGE == 11:
            # between-slot interleave points only (no within-slot emissions)
            for g in range(4):
                stats_group(g)
            for t in range(16):
                ln_tile(t)
            for g in range(4):
                k_group(g)
                q_slot(g)
            for hp in range(4):
                att_head_pair(0, 2 * hp)
            for hp in range(4):
                att_head_pair(1, 2 * hp)
            epilogue_proj(0); epilogue_proj(1); ln_rstd(mvs2, rst2, 0, 2)
            epilogue_ln2(0); epilogue_ln2(1)
            for hp in range(4):
                att_head_pair(2, 2 * hp)
            epilogue_proj(2); epilogue_proj(3); ln_rstd(mvs2, rst2, 2, 2)
            epilogue_ln2(2); epilogue_ln2(3)
            ffn_chunk(0)
            for hp in range(4):
                att_head_pair(3, 2 * hp)
            epilogue_proj(4); epilogue_proj(5); ln_rstd(mvs2, rst2, 4, 2)
            epilogue_ln2(4); epilogue_ln2(5)
            ffn_chunk(1)
            epilogue_proj(6); epilogue_proj(7); ln_rstd(mvs2, rst2, 6, 2)
            epilogue_ln2(6); epilogue_ln2(7)
            ffn_chunk(2)
            ffn_chunk(3)
        elif STAGE == 10:
            # QKV upfront; epilogue/FFN interleaved after each slot and
            # between slot-3 pairs
            for g in range(4):
                stats_group(g)
            for t in range(16):
                ln_tile(t)
            for g in range(4):
                k_group(g)
                q_slot(g)
            for hp in range(4):
                att_head_pair(0, 2 * hp)
            for hp in range(4):
                att_head_pair(1, 2 * hp)
            epilogue_proj(0); epilogue_proj(1); ln_rstd(mvs2, rst2, 0, 2)
            epilogue_ln2(0); epilogue_ln2(1)
            att_head_pair(2, 0)
            att_head_pair(2, 2)
            epilogue_proj(2); epilogue_proj(3); ln_rstd(mvs2, rst2, 2, 2)
            att_head_pair(2, 4)
            epilogue_ln2(2); epilogue_ln2(3)
            att_head_pair(2, 6)
            ffn_chunk(0)
            att_head_pair(3, 0)
            epilogue_proj(4); epilogue_proj(5); ln_rstd(mvs2, rst2, 4, 2)
            att_head_pair(3, 2)
            epilogue_ln2(4); epilogue_ln2(5)
            att_head_pair(3, 4)
            ffn_chunk(1)
            att_head_pair(3, 6)
            ffn_chunk(2)
            epilogue_proj(6); epilogue_proj(7); ln_rstd(mvs2, rst2, 6, 2)
            epilogue_ln2(6); epilogue_ln2(7)
            ffn_chunk(3)
        elif STAGE == 8:
            # stage 4 plus ONE interleave point: slot 0-1 epilogues before slot3
            for g in range(4):
                stats_group(g)
            for t in range(16):
                ln_tile(t)
            for g in range(4):
                k_group(g)
                q_slot(g)
            for s in range(3):
                for hp in range(4):
                    att_head_pair(s, 2 * hp)
            for t in range(4):
                epilogue_proj(t)
                if t % 2 == 1:
                    ln_rstd(mvs2, rst2, t - 1, 2)
            for t in range(4):
                epilogue_ln2(t)
            for hp in range(4):
                att_head_pair(3, 2 * hp)
            for t in range(4, OWN_NT):
                epilogue_proj(t)
                if t % 2 == 1:
                    ln_rstd(mvs2, rst2, t - 1, 2)
            for t in range(4, OWN_NT):
                epilogue_ln2(t)
            for c in range(4):
                ffn_chunk(c)
        elif STAGE == 7:
            # QKV fully upfront; epilogue/FFN woven intra-pair into slots 2-3
            for g in range(4):
                stats_group(g)
            for t in range(16):
                ln_tile(t)
            for g in range(4):
                k_group(g)
                q_slot(g)
            for s in range(2):
                for hp in range(4):
                    att_head_pair(s, 2 * hp)
            att_head_pair(2, 0)
            att_head_pair(2, 2)
            att_head_pair(2, 4, [lambda: epilogue_proj(0),
                                 lambda: epilogue_proj(1),
                                 lambda: ln_rstd(mvs2, rst2, 0, 2)])
            att_head_pair(2, 6, [lambda: epilogue_ln2(0),
                                 lambda: epilogue_ln2(1),
                                 lambda: epilogue_proj(2)])
            f0 = ffn_pieces(0)
            att_head_pair(3, 0, [lambda: (epilogue_proj(3),
                                          ln_rstd(mvs2, rst2, 2, 2)),
                                 lambda: epilogue_ln2(2),
                                 lambda: epilogue_ln2(3),
                                 f0[0], f0[1], f0[2]])
            f1p = ffn_pieces(1)
            att_head_pair(3, 2, [f0[3], f0[4], f0[5], f0[6],
                                 lambda: epilogue_proj(4),
                                 lambda: (epilogue_proj(5),
                                          ln_rstd(mvs2, rst2, 4, 2)),
                                 f1p[0], f1p[1]])
            f2 = ffn_pieces(2)
            att_head_pair(3, 4, [f1p[2], f1p[3], f1p[4], f1p[5], f1p[6],
                                 lambda: epilogue_ln2(4),
                                 lambda: epilogue_ln2(5), f2[0]])
            att_head_pair(3, 6, [f2[1], f2[2], f2[3], f2[4], f2[5], f2[6]])
            epilogue_proj(6)
            epilogue_proj(7)
            ln_rstd(mvs2, rst2, 6, 2)
            epilogue_ln2(6)
            epilogue_ln2(7)
            ffn_chunk(3)
        elif STAGE == 6:
            # between-pair interleaving (no intra-pair weaving)
            stats_group(0)
            for t in range(4):
                ln_tile(t)
            k_group(0, (0, 1))
            q_slot(0, (0, 1))
            att_head_pair(0, 0)
            stats_group(1); ln_tile(4)
            att_head_pair(0, 2)
            ln_tile(5); kq(0, 2)()
            att_head_pair(0, 4)
            ln_tile(6); ln_tile(7); kq(0, 3)()
            att_head_pair(0, 6)
            kq(1, 0)(); kq(1, 1)()
            att_head_pair(1, 0)
            stats_group(2); ln_tile(8); kq(1, 2)()
            att_head_pair(1, 2)
            ln_tile(9); kq(1, 3)()
            att_head_pair(1, 4)
            ln_tile(10); ln_tile(11); kq(2, 0)()
            att_head_pair(1, 6)
            kq(2, 1)(); stats_group(3); kq(2, 2)()
            att_head_pair(2, 0)
            ln_tile(12); ln_tile(13); kq(2, 3)()
            att_head_pair(2, 2)
            ln_tile(14); ln_tile(15)
            att_head_pair(2, 4)
            epilogue_proj(0); epilogue_proj(1); ln_rstd(mvs2, rst2, 0, 2)
            att_head_pair(2, 6)
            epilogue_ln2(0); epilogue_ln2(1); kq(3, 0)(); kq(3, 1)()
            att_head_pair(3, 0)
            kq(3, 2)(); kq(3, 3)()
            epilogue_proj(2); epilogue_proj(3); ln_rstd(mvs2, rst2, 2, 2)
            att_head_pair(3, 2)
            epilogue_ln2(2); epilogue_ln2(3)
            ffn_chunk(0)
            att_head_pair(3, 4)
            ffn_chunk(1)
            epilogue_proj(4); epilogue_proj(5); ln_rstd(mvs2, rst2, 4, 2)
            att_head_pair(3, 6)
            epilogue_ln2(4); epilogue_ln2(5)
            ffn_chunk(2)
            epilogue_proj(6); epilogue_proj(7); ln_rstd(mvs2, rst2, 6, 2)
            epilogue_ln2(6); epilogue_ln2(7)
            ffn_chunk(3)
        elif STAGE == 5:
            # full ln/kq weave; epilogues+FFN sequential
            stats_group(0)
            for t in range(4):
                ln_tile(t)
            k_group(0, (0, 1))
            q_slot(0, (0, 1))
            att_head_pair(0, 0, [lambda: stats_group(1), lambda: ln_tile(4)])
            att_head_pair(0, 2, [lambda: ln_tile(5), kq(0, 2)])
            att_head_pair(0, 4, [lambda: ln_tile(6), lambda: ln_tile(7),
                                 kq(0, 3)])
            att_head_pair(0, 6, [kq(1, 0), kq(1, 1)])
            att_head_pair(1, 0, [lambda: stats_group(2), lambda: ln_tile(8),
                                 kq(1, 2)])
            att_head_pair(1, 2, [lambda: ln_tile(9), kq(1, 3)])
            att_head_pair(1, 4, [lambda: ln_tile(10), lambda: ln_tile(11),
                                 kq(2, 0)])
            att_head_pair(1, 6, [kq(2, 1), lambda: stats_group(3), kq(2, 2)])
            att_head_pair(2, 0, [lambda: ln_tile(12), kq(2, 3),
                                 lambda: ln_tile(13)])
            att_head_pair(2, 2, [lambda: ln_tile(14), lambda: ln_tile(15)])
            att_head_pair(2, 4, [kq(3, 0), kq(3, 1)])
            att_head_pair(2, 6, [kq(3, 2), kq(3, 3)])
            for hp in range(4):
                att_head_pair(3, 2 * hp)
            for t in range(OWN_NT):
                epilogue_proj(t)
                if t % 2 == 1:
                    ln_rstd(mvs2, rst2, t - 1, 2)
            for t in range(OWN_NT):
                epilogue_ln2(t)
            for c in range(4):
                ffn_chunk(c)
        elif STAGE <= 4:
            # reduced program for HW bisection
            for g in range(4):
                stats_group(g)
            for t in range(16):
                ln_tile(t)
            for g in range(4):
                k_group(g)
                q_slot(g)
            if STAGE >= 2:
                for s in range(4):
                    for hp in range(4):
                        att_head_pair(s, 2 * hp)
            if STAGE >= 3:
                for t in range(OWN_NT):
                    epilogue_proj(t)
                    if t % 2 == 1:
                        ln_rstd(mvs2, rst2, t - 1, 2)
                for t in range(OWN_NT):
                    epilogue_ln2(t)
            if STAGE >= 4:
                for c in range(4):
                    ffn_chunk(c)
            else:
                for t in range(OWN_NT):
                    o_t = opool.tile([128, D], FP32, tag="ot", name="ot")
                    nc.vector.tensor_copy(
                        o_t[:], (x2 if STAGE >= 3 else xb)[t][:])
                    nc.sync.dma_start(out_dram[t * 128:(t + 1) * 128, :],
                                      o_t[:])
        else:
            _full_schedule()

    import os
    if not os.environ.get("SKIP_WAITFIX"):
        _split_multi_waits(nc)
    return nc


_NC_CACHE = None


def _get_nc():
    global _NC_CACHE
    if _NC_CACHE is None:
        _NC_CACHE = _build_program()
    return _NC_CACHE


# ---------------------------------------------------------------------------
# Host side
# ---------------------------------------------------------------------------
def _to_fp8(a):
    return np.clip(a, -240.0, 240.0).astype(F8NP)


def _pack_dr(W):
    """[512, N] -> [2, 128, 2, N] fp8 with W scaled by WS."""
    Ws = np.asarray(W, np.float64) * WS
    out = np.empty((2, 128, 2, Ws.shape[1]), np.float64)
    for kp in range(2):
        for j in range(2):
            out[kp, :, j, :] = Ws[kp * 256 + j * 128:kp * 256 + (j + 1) * 128, :]
    return _to_fp8(out)


def _fold_weights(Wq, bq, Wk, bk, Wv, bv, Wp, bp, W1, b1, W2, b2, g1, be1,
                  g2, be2):
    f64 = np.float64
    Wq_e = (g1.astype(f64)[None, :, None] * Wq.astype(f64))      # [H,D,DK]
    Wk_e = (g1.astype(f64)[None, :, None] * Wk.astype(f64))
    Wv_e = (g1.astype(f64)[None, :, None] * Wv.astype(f64))
    bq_e = bq.astype(f64) + np.einsum("d,hdk->hk", be1.astype(f64), Wq.astype(f64))
    bk_e = bk.astype(f64) + np.einsum("d,hdk->hk", be1.astype(f64), Wk.astype(f64))
    bv_e = bv.astype(f64) + np.einsum("d,hdk->hk", be1.astype(f64), Wv.astype(f64))
    W1_e = g2.astype(f64)[:, None] * W1.astype(f64)
    b1_e = b1.astype(f64) + be2.astype(f64) @ W1.astype(f64)

    def head_major(W):  # [H,D,DK] -> [D, H*DK]
        return np.transpose(W, (1, 0, 2)).reshape(D, H * DK)

    out = {}
    out["wq8"] = _pack_dr(head_major(Wq_e))
    out["wk8"] = _pack_dr(head_major(Wk_e))
    out["wv8"] = _pack_dr(head_major(Wv_e))
    # wp8: quad g covers heads 4g..4g+3; [128(2 heads x 64k), 2(head pair), D]
    Wp_f = Wp.astype(f64).reshape(H, DK, D)
    wp8 = np.empty((2, 128, 2, D), np.float64)
    for g in range(2):
        for j in range(2):
            h0 = 4 * g + 2 * j
            wp8[g, 0:64, j, :] = Wp_f[h0] * WS
            wp8[g, 64:128, j, :] = Wp_f[h0 + 1] * WS
    out["wp8"] = _to_fp8(wp8)
    out["w18"] = _pack_dr(W1_e)
    # w28: [8 hidpair, 128, 2, D]
    W2_f = W2.astype(f64) * WS
    w28 = np.empty((8, 128, 2, D), np.float64)
    for hp in range(8):
        for j in range(2):
            w28[hp, :, j, :] = W2_f[hp * 256 + j * 128:hp * 256 + (j + 1) * 128, :]
    out["w28"] = _to_fp8(w28)

    bqk = np.zeros((128, 8), np.float32)
    for pr in range(4):
        bqk[:, pr] = np.concatenate([bq_e[2 * pr], bq_e[2 * pr + 1]])
        bqk[:, 4 + pr] = np.concatenate([bk_e[2 * pr], bk_e[2 * pr + 1]])
    out["bqk"] = bqk
    out["b1s"] = np.ascontiguousarray(
        (WS * b1_e).reshape(16, 128).T).astype(np.float32)
    out["bv_row"] = bv_e.reshape(1, H * DK).astype(np.float32)
    out["bp_row"] = bp.reshape(1, D).astype(np.float32)
    out["b2_row"] = b2.reshape(1, D).astype(np.float32)
    return out


def _build_masks(p):
    """[4, 128, 2, 1024] fp8: per slot, DR rhs for the last quad's mask."""
    perm = PERM[p]
    masks = np.zeros((4, 128, 2, 1024), np.float32)
    for s in range(4):
        U = SPANS[s]
        qc = perm[OWN_POS[s]]              # original own chunk for this slot
        q_tok = qc * 256 + np.arange(256)  # original q token ids
        for j in range(4):
            u = (U - 4) + j                # x' kv tile index
            kv_tok = perm[u // 2] * 256 + (u % 2) * 128 + np.arange(128)
            m = np.where(kv_tok[:, None] <= q_tok[None, :], 0.0, NEG)
            masks[s, :, 0, j * 256:(j + 1) * 256] = m
    return masks.astype(F8NP)


def kernel(x, Wq, bq, Wk, bk, Wv, bv, Wp, bp, W1, b1, W2, b2, g1, be1, g2, be2):
    x = np.asarray(x, np.float32)
    folded = _fold_weights(
        np.asarray(Wq), np.asarray(bq), np.asarray(Wk), np.asarray(bk),
        np.asarray(Wv), np.asarray(bv), np.asarray(Wp), np.asarray(bp),
        np.asarray(W1), np.asarray(b1), np.asarray(W2), np.asarray(b2),
        np.asarray(g1), np.asarray(be1), np.asarray(g2), np.asarray(be2))

    masks_by_p = [_build_masks(0), _build_masks(1)]
    in_maps = []
    for c in range(8):
        b, p = c // 2, c % 2
        perm = PERM[p]
        xp = np.concatenate([x[b, pc * 256:(pc + 1) * 256] for pc in perm])
        m = dict(folded)
        m["xp"] = np.ascontiguousarray(xp)
        m["masks8"] = masks_by_p[p]
        in_maps.append(m)

    nc = _get_nc()
    res = run_bass_kernel_spmd(nc, in_maps, list(range(8)))

    out = np.empty((B, T, D), np.float32)
    for c in range(8):
        b, p = c // 2, c % 2
        perm = PERM[p]
        o = res.results[c]["out"]
        for s in range(4):
            oc = perm[OWN_POS[s]]
            out[b, oc * 256:(oc + 1) * 256] = o[s * 256:(s + 1) * 256]
    return out
